# revision 12
# baseline (speedup 1.0000x reference)
"""GCN encoder (2x GCNConv + mean-pool + two linear heads) on 8 NeuronCores.

Strategy (graph/data parallel, per sharding hint):
 - Nodes are range-partitioned across the 8 cores at graph boundaries
   (so global mean-pool is core-local). Each core owns the incident
   edges of its dst nodes (plus self-loops as explicit edges).
 - Layer l: every core redundantly computes the full transform table
   t = h @ W (dense matmul, cheap), then gathers t[src] rows for its own
   edges with GPSIMD dma_gather and segment-sums them per 128-node dst
   block on the TensorEngine via an on-chip-built selection matrix
   S[e, d] = coef_e * (dstlocal_e == d)   (one DVE tensor_scalar op).
 - h1 is exchanged between the two layers through the host (two NEFF
   launches; the host concatenates the 8 shards and feeds h1^T back),
   which is cheaper here than the ncfw AllGather (~110us floor).
 - Pooling + the two Z=32 projections run on-device in launch 2.

The gather index stream is int16 against a table base centered at row
32768 (signed descriptor offsets cover all 50176 rows); the last index
of each 1024-edge batch must be >= 0 (ucode trims trailing negatives),
arranged by an in-chunk swap on the host.
"""
import sys, os
sys.path.insert(0, "/opt/trn_rl_repo")
import numpy as np
import ml_dtypes

import concourse.bacc as bacc
import concourse.tile as tile
import concourse.mybir as mybir
from concourse.bass_utils import run_bass_kernel_spmd

N, F, H, Z, G = 50000, 128, 64, 32, 512
NC = 8
RPAD = 50176           # 392 * 128, padded table rows
BASE = 32768           # gather base row (signed int16 window covers [0, 65535])
CHB = 8                # chunks per gather batch (1024 edges)
NQ = 4                 # SWDGE queues (parallel Q7 descriptor generation)

_dt = mybir.dt


# ----------------------------------------------------------------- host prep

def _wrap_idx_batches(idx16):
    """[NBATCH*1024] int16 -> [NBATCH, 128, 64] wrapped+replicated layout."""
    nb = idx16.shape[0] // (CHB * 128)
    il = idx16.reshape(nb, CHB * 128)
    lanes = np.arange(CHB * 128)
    out = np.zeros((nb, 128, CHB * 128 // 16), dtype=np.int16)
    for grp in range(8):
        out[:, grp * 16 + lanes % 16, lanes // 16] = il
    return out


def preprocess(edge_index, batch):
    src = np.asarray(edge_index[0], dtype=np.int64)
    dst = np.asarray(edge_index[1], dtype=np.int64)
    batch = np.asarray(batch, dtype=np.int64)
    deg = np.bincount(dst, minlength=N).astype(np.float64) + 1.0
    dinv = 1.0 / np.sqrt(deg)
    coef = (dinv[src] * dinv[dst]).astype(np.float32)
    srcA = np.concatenate([src, np.arange(N)])
    dstA = np.concatenate([dst, np.arange(N)])
    coefA = np.concatenate([coef, (dinv * dinv).astype(np.float32)])

    gcnt = np.bincount(batch, minlength=G)
    gcum = np.concatenate([[0], np.cumsum(gcnt)])  # node index where graph g starts
    bounds = [0]
    for c in range(1, NC):
        gi = int(np.argmin(np.abs(gcum - round(c * N / NC))))
        bounds.append(int(gcum[gi]))
    bounds.append(N)
    gbounds = [int(np.searchsorted(gcum, b)) for b in bounds]  # graph index bounds

    cores = []
    for c in range(NC):
        s, e = bounds[c], bounds[c + 1]
        m = (dstA >= s) & (dstA < e)
        es, ed, ec = srcA[m], dstA[m], coefA[m]
        blk = (ed - s) // 128
        nblk = -(-(e - s) // 128)
        order = np.argsort(blk, kind="stable")
        es, ed, ec, blk = es[order], ed[order], ec[order], blk[order]
        cnt = np.bincount(blk, minlength=nblk)
        chunks = (cnt + 127) // 128
        perm = np.argsort(-chunks, kind="stable")  # block processing order
        cores.append(dict(s=s, e=e, es=es, ed=ed, ec=ec, cnt=cnt,
                          chunks=chunks, perm=perm, nblk=nblk,
                          gs=gbounds[c], ge=gbounds[c + 1]))

    B_slots = max(cd["nblk"] for cd in cores)
    Kj = np.ones(B_slots, dtype=np.int64)
    for cd in cores:
        ch = cd["chunks"][cd["perm"]]
        Kj[: cd["nblk"]] = np.maximum(Kj[: cd["nblk"]], ch)
    TOTCH = int(Kj.sum())
    TOT_PAD = -(-TOTCH // CHB) * CHB
    NBATCH = TOT_PAD // CHB
    Cj = np.concatenate([[0], np.cumsum(Kj)])  # chunk start per slot

    sched = dict(B_slots=B_slots, Kj=Kj, Cj=Cj, TOT_PAD=TOT_PAD, NBATCH=NBATCH)

    for cd in cores:
        NEdge = TOT_PAD * 128
        idx16 = np.zeros(NEdge, dtype=np.int16)
        dloc = np.full(NEdge, -1.0, dtype=np.float32)
        cf = np.zeros(NEdge, dtype=np.float32)
        estart = np.concatenate([[0], np.cumsum(cd["cnt"])])
        for j in range(cd["nblk"]):
            b = cd["perm"][j]
            k = int(cd["cnt"][b])
            if k == 0:
                continue
            e0 = int(estart[b])
            pos = Cj[j] * 128 + np.arange(k)
            idx16[pos] = (cd["es"][e0 : e0 + k] - BASE).astype(np.int16)
            dloc[pos] = (cd["ed"][e0 : e0 + k] - (cd["s"] + b * 128)).astype(np.float32)
            cf[pos] = cd["ec"][e0 : e0 + k]
        # last index of each 1024-batch must be >= 0: swap inside last chunk
        for bidx in range(NBATCH):
            last = bidx * 1024 + 1023
            if idx16[last] < 0:
                lo = bidx * 1024 + 896
                cand = np.nonzero(idx16[lo : last + 1] >= 0)[0]
                assert len(cand), "all-negative chunk: need cross-chunk swap"
                p = lo + cand[0]
                for a in (idx16, dloc, cf):
                    a[last], a[p] = a[p], a[last]
        cd["idx_w"] = _wrap_idx_batches(idx16)
        cd["dloc_t"] = dloc.reshape(TOT_PAD, 128).T.copy()
        cd["cf_t"] = cf.reshape(TOT_PAD, 128).T.copy()
        cd["idx16"] = idx16
        cd["dloc"] = dloc
        cd["cf"] = cf

        # pooling: graph-local id per (slot, lane); -1 on pads
        bloc = np.full((128, B_slots), -1.0, dtype=np.float32)
        for j in range(cd["nblk"]):
            b = cd["perm"][j]
            lo = cd["s"] + b * 128
            hi = min(lo + 128, cd["e"])
            lanes = np.arange(hi - lo)
            bloc[lanes, j] = (batch[lo:hi] - cd["gs"]).astype(np.float32)
        cd["bloc"] = bloc
        ic = np.zeros((128, 1), dtype=np.float32)
        ngr = cd["ge"] - cd["gs"]
        ic[:ngr, 0] = 1.0 / np.maximum(gcnt[cd["gs"] : cd["ge"]], 1.0)
        cd["invcnt"] = ic

    return cores, sched


# ------------------------------------------------------------- bass builders

GRP = 8  # node blocks per transform group

def _emit_table_transform(nc, tc, pools, table, hT, W_t, ntiles, kdim):
    """table rows (bf16, 128-col padded) = hT.T @ W; batched loads/writes."""
    for g0 in range(0, ntiles, GRP):
        lt = pools["xt"].tile([kdim, GRP * 128], _dt.bfloat16, tag="xt")
        nc.sync.dma_start(lt[:], hT[:kdim, g0 * 128 : (g0 + GRP) * 128])
        ct = pools["tout"].tile([128, GRP, H], _dt.bfloat16, tag="tout")
        for k in range(GRP):
            ps = pools["tps"].tile([128, H], _dt.float32, tag="tps")
            nc.tensor.matmul(ps[:], lhsT=lt[:, k * 128 : (k + 1) * 128],
                             rhs=W_t[:kdim, :], start=True, stop=True)
            nc.vector.tensor_copy(ct[:, k, :], ps[:])
        out_view = table[g0 * 128 : (g0 + GRP) * 128, 0:H].rearrange(
            "(k p) c -> p k c", k=GRP)
        nc.sync.dma_start(out_view, ct[:])


PRE_ST = 144  # selection matrices built before the table barrier

def _emit_st_prebuild(nc, pools, dloc_t, iota_t, sched, npre=PRE_ST):
    """Build the first PRE_ST chunk selection matrices (no table dep)."""
    pre = {}
    n = min(npre, sched["TOT_PAD"])
    for cglob in range(n):
        st = pools["st"].tile([128, 128], _dt.bfloat16, tag="st")
        nc.vector.tensor_scalar(
            out=st[:], in0=iota_t[:],
            scalar1=dloc_t[:, cglob : cglob + 1], scalar2=None,
            op0=mybir.AluOpType.is_equal,
        )
        pre[cglob] = st
    return pre


def _emit_gather_agg(nc, tc, pools, table, idx_in, dloc_t, cf_t, iota_t, sched,
                     slot_epilogue, pre_st=None):
    """Gather stream + per-slot matmul segment-sum. slot_epilogue(j, psum)."""
    B_slots, Kj, Cj, NBATCH = sched["B_slots"], sched["Kj"], sched["Cj"], sched["NBATCH"]
    gtiles = {}
    pre_st = pre_st or {}

    def ensure_batch(b):
        if b in gtiles or b >= NBATCH:
            return
        it = pools["idx"].tile([128, CHB * 128 // 16], _dt.int16, tag="idx")
        nc.sync.dma_start(it[:], idx_in[b])
        g = pools["g"].tile([128, CHB, 128], _dt.bfloat16, tag="g")
        nc.gpsimd.dma_gather(
            out_ap=g[:], in_ap=table[BASE:, :], idxs_ap=it[:],
            num_idxs=CHB * 128, num_idxs_reg=CHB * 128, elem_size=128,
            queue_num=b % NQ,
        )
        gtiles[b] = g
        if len(gtiles) > 128:  # drop old refs (pool recycles anyway)
            del gtiles[min(gtiles)]

    ensure_batch(0)
    ensure_batch(1)
    for j in range(B_slots):
        ps = pools["aps"].tile([128, H], _dt.float32, tag="aps")
        k0, k1 = int(Cj[j]), int(Cj[j + 1])
        for cglob in range(k0, k1):
            b, col = cglob // CHB, cglob % CHB
            ensure_batch(b)
            for la in range(1, 24):
                ensure_batch(b + la)
            ms = pools["ms"].tile([128, H], _dt.bfloat16, tag="ms")
            nc.scalar.activation(ms[:], gtiles[b][:, col, 0:H],
                                 mybir.ActivationFunctionType.Copy,
                                 scale=cf_t[:, cglob : cglob + 1])
            if cglob in pre_st:
                st = pre_st.pop(cglob)
            else:
                st = pools["st"].tile([128, 128], _dt.bfloat16, tag="st")
                nc.vector.tensor_scalar(
                    out=st[:], in0=iota_t[:],
                    scalar1=dloc_t[:, cglob : cglob + 1], scalar2=None,
                    op0=mybir.AluOpType.is_equal,
                )
            nc.tensor.matmul(ps[:], lhsT=st[:], rhs=ms[:],
                             start=(cglob == k0), stop=(cglob == k1 - 1))
        slot_epilogue(j, ps)


def build_launch1(sched):
    nc = bacc.Bacc("TRN2", debug=False, num_devices=NC, num_swdge_queues=NQ)
    B_slots, NBATCH, TOT_PAD = sched["B_slots"], sched["NBATCH"], sched["TOT_PAD"]

    xT = nc.dram_tensor("xT", [F, RPAD], _dt.bfloat16, kind="ExternalInput")
    W1 = nc.dram_tensor("W1", [F, H], _dt.bfloat16, kind="ExternalInput")
    B1 = nc.dram_tensor("B1", [128, H], _dt.float32, kind="ExternalInput")
    iota = nc.dram_tensor("iota", [128, 128], _dt.bfloat16, kind="ExternalInput")
    idx = nc.dram_tensor("idx", [NBATCH, 128, CHB * 128 // 16], _dt.int16, kind="ExternalInput")
    dloc = nc.dram_tensor("dloc", [128, TOT_PAD], _dt.float32, kind="ExternalInput")
    cf = nc.dram_tensor("cf", [128, TOT_PAD], _dt.float32, kind="ExternalInput")
    h1o = nc.dram_tensor("h1o", [B_slots * 128, H], _dt.float32, kind="ExternalOutput")
    t1 = nc.dram_tensor("t1", [RPAD, 128], _dt.bfloat16)

    with tile.TileContext(nc) as tc:
        with (
            tc.tile_pool(name="consts", bufs=1) as consts,
            tc.tile_pool(name="xt", bufs=3) as xt_p,
            tc.tile_pool(name="tout", bufs=4) as tout_p,
            tc.tile_pool(name="idx", bufs=32) as idx_p,
            tc.tile_pool(name="g", bufs=32) as g_p,
            tc.tile_pool(name="st", bufs=PRE_ST + 10) as st_p,
            tc.tile_pool(name="ms", bufs=10) as ms_p,
            tc.tile_pool(name="ho", bufs=4) as ho_p,
            tc.tile_pool(name="tps", bufs=2, space="PSUM") as tps_p,
            tc.tile_pool(name="aps", bufs=6, space="PSUM") as aps_p,
        ):
            pools = dict(xt=xt_p, tout=tout_p, idx=idx_p, g=g_p, st=st_p,
                         ms=ms_p, tps=tps_p, aps=aps_p)
            w1_t = consts.tile([F, H], _dt.bfloat16)
            nc.sync.dma_start(w1_t[:], W1[:, :])
            b1_t = consts.tile([128, H], _dt.float32)
            nc.sync.dma_start(b1_t[:], B1[:, :])
            iota_t = consts.tile([128, 128], _dt.bfloat16)
            nc.sync.dma_start(iota_t[:], iota[:, :])
            dloc_t = consts.tile([128, TOT_PAD], _dt.float32)
            nc.sync.dma_start(dloc_t[:], dloc[:, :])
            cf_t = consts.tile([128, TOT_PAD], _dt.float32)
            nc.sync.dma_start(cf_t[:], cf[:, :])

            pre_st = _emit_st_prebuild(nc, pools, dloc_t, iota_t, sched)
            _emit_table_transform(nc, tc, pools, t1, xT, w1_t, RPAD // 128, F)
            tc.strict_bb_all_engine_barrier()

            def epi(j, ps):
                hb = ho_p.tile([128, H], _dt.float32, tag="ho")
                nc.vector.tensor_tensor(out=hb[:], in0=ps[:], in1=b1_t[:],
                                        op=mybir.AluOpType.add)
                nc.vector.tensor_scalar_max(hb[:], hb[:], 0.0)
                nc.sync.dma_start(h1o[j * 128 : (j + 1) * 128, :], hb[:])

            _emit_gather_agg(nc, tc, pools, t1, idx, dloc_t, cf_t, iota_t, sched, epi, pre_st)
    nc.finalize()
    return nc


def build_launch2(sched):
    nc = bacc.Bacc("TRN2", debug=False, num_devices=NC, num_swdge_queues=NQ)
    B_slots, NBATCH, TOT_PAD = sched["B_slots"], sched["NBATCH"], sched["TOT_PAD"]

    h1T = nc.dram_tensor("h1T", [H, RPAD], _dt.bfloat16, kind="ExternalInput")
    W2 = nc.dram_tensor("W2", [H, H], _dt.bfloat16, kind="ExternalInput")
    B2 = nc.dram_tensor("B2", [128, H], _dt.float32, kind="ExternalInput")
    iota = nc.dram_tensor("iota", [128, 128], _dt.bfloat16, kind="ExternalInput")
    idx = nc.dram_tensor("idx", [NBATCH, 128, CHB * 128 // 16], _dt.int16, kind="ExternalInput")
    dloc = nc.dram_tensor("dloc", [128, TOT_PAD], _dt.float32, kind="ExternalInput")
    cf = nc.dram_tensor("cf", [128, TOT_PAD], _dt.float32, kind="ExternalInput")
    bloc = nc.dram_tensor("bloc", [128, B_slots], _dt.float32, kind="ExternalInput")
    iotaG = nc.dram_tensor("iotaG", [128, 128], _dt.float32, kind="ExternalInput")
    invc = nc.dram_tensor("invc", [128, 1], _dt.float32, kind="ExternalInput")
    ident = nc.dram_tensor("ident", [128, 128], _dt.float32, kind="ExternalInput")
    Wmu = nc.dram_tensor("Wmu", [H, Z], _dt.float32, kind="ExternalInput")
    Wls = nc.dram_tensor("Wls", [H, Z], _dt.float32, kind="ExternalInput")
    Bmu = nc.dram_tensor("Bmu", [128, Z], _dt.float32, kind="ExternalInput")
    Bls = nc.dram_tensor("Bls", [128, Z], _dt.float32, kind="ExternalInput")
    muo = nc.dram_tensor("muo", [128, Z], _dt.float32, kind="ExternalOutput")
    lso = nc.dram_tensor("lso", [128, Z], _dt.float32, kind="ExternalOutput")
    t2 = nc.dram_tensor("t2", [RPAD, 128], _dt.bfloat16)

    with tile.TileContext(nc) as tc:
        with (
            tc.tile_pool(name="consts", bufs=1) as consts,
            tc.tile_pool(name="xt", bufs=3) as xt_p,
            tc.tile_pool(name="tout", bufs=4) as tout_p,
            tc.tile_pool(name="idx", bufs=32) as idx_p,
            tc.tile_pool(name="g", bufs=32) as g_p,
            tc.tile_pool(name="st", bufs=PRE_ST + 10) as st_p,
            tc.tile_pool(name="ms", bufs=10) as ms_p,
            tc.tile_pool(name="h2", bufs=4) as h2_p,
            tc.tile_pool(name="sg", bufs=4) as sg_p,
            tc.tile_pool(name="fin", bufs=8) as fin_p,
            tc.tile_pool(name="tps", bufs=2, space="PSUM") as tps_p,
            tc.tile_pool(name="aps", bufs=3, space="PSUM") as aps_p,
            tc.tile_pool(name="pps", bufs=1, space="PSUM") as pps_p,
            tc.tile_pool(name="fps", bufs=1, space="PSUM") as fps_p,
        ):
            pools = dict(xt=xt_p, tout=tout_p, idx=idx_p, g=g_p, st=st_p,
                         ms=ms_p, tps=tps_p, aps=aps_p)
            w2_t = consts.tile([H, H], _dt.bfloat16)
            nc.sync.dma_start(w2_t[:], W2[:, :])
            b2_t = consts.tile([128, H], _dt.float32)
            nc.sync.dma_start(b2_t[:], B2[:, :])
            iota_t = consts.tile([128, 128], _dt.bfloat16)
            nc.sync.dma_start(iota_t[:], iota[:, :])
            dloc_t = consts.tile([128, TOT_PAD], _dt.float32)
            nc.sync.dma_start(dloc_t[:], dloc[:, :])
            cf_t = consts.tile([128, TOT_PAD], _dt.float32)
            nc.sync.dma_start(cf_t[:], cf[:, :])
            bloc_t = consts.tile([128, B_slots], _dt.float32)
            nc.sync.dma_start(bloc_t[:], bloc[:, :])
            iog_t = consts.tile([128, 128], _dt.float32)
            nc.sync.dma_start(iog_t[:], iotaG[:, :])
            invc_t = consts.tile([128, 1], _dt.float32)
            nc.sync.dma_start(invc_t[:], invc[:, :])
            id_t = consts.tile([128, 128], _dt.float32)
            nc.sync.dma_start(id_t[:], ident[:, :])
            wmu_t = consts.tile([H, Z], _dt.float32)
            nc.sync.dma_start(wmu_t[:], Wmu[:, :])
            wls_t = consts.tile([H, Z], _dt.float32)
            nc.sync.dma_start(wls_t[:], Wls[:, :])
            bmu_t = consts.tile([128, Z], _dt.float32)
            nc.sync.dma_start(bmu_t[:], Bmu[:, :])
            bls_t = consts.tile([128, Z], _dt.float32)
            nc.sync.dma_start(bls_t[:], Bls[:, :])

            pre_st = _emit_st_prebuild(nc, pools, dloc_t, iota_t, sched, 96)
            _emit_table_transform(nc, tc, pools, t2, h1T, w2_t, RPAD // 128, H)
            tc.strict_bb_all_engine_barrier()

            pool_ps = pps_p.tile([128, H], _dt.float32)

            def epi(j, ps):
                hb = h2_p.tile([128, H], _dt.float32, tag="h2")
                nc.vector.tensor_tensor(out=hb[:], in0=ps[:], in1=b2_t[:],
                                        op=mybir.AluOpType.add)
                nc.vector.tensor_scalar_max(hb[:], hb[:], 0.0)
                sg = sg_p.tile([128, 128], _dt.float32, tag="sg")
                nc.vector.tensor_scalar(
                    out=sg[:], in0=iog_t[:],
                    scalar1=bloc_t[:, j : j + 1], scalar2=None,
                    op0=mybir.AluOpType.is_equal,
                )
                nc.tensor.matmul(pool_ps[:], lhsT=sg[:], rhs=hb[:],
                                 start=(j == 0), stop=(j == B_slots - 1))

            _emit_gather_agg(nc, tc, pools, t2, idx, dloc_t, cf_t, iota_t, sched, epi, pre_st)

            pooled = fin_p.tile([128, H], _dt.float32, tag="pooled")
            nc.vector.tensor_scalar_mul(pooled[:], pool_ps[:], invc_t[:, 0:1])
            ptp = fps_p.tile([H, 128], _dt.float32, tag="ptp")
            nc.tensor.transpose(ptp[:], pooled[:], id_t[:])
            pooledT = fin_p.tile([H, 128], _dt.float32, tag="pooledT")
            nc.vector.tensor_copy(pooledT[:], ptp[:])
            for wt, bt, oo in ((wmu_t, bmu_t, muo), (wls_t, bls_t, lso)):
                ops = fps_p.tile([128, Z], _dt.float32, tag="ops")
                nc.tensor.matmul(ops[:], lhsT=pooledT[:], rhs=wt[:], start=True, stop=True)
                ot = fin_p.tile([128, Z], _dt.float32, tag="ot")
                nc.vector.tensor_tensor(out=ot[:], in0=ops[:], in1=bt[:],
                                        op=mybir.AluOpType.add)
                nc.sync.dma_start(oo[:, :], ot[:])
    nc.finalize()
    return nc


# ------------------------------------------------------------------- runner

_cache = {}


def _get_programs(sched):
    key = (sched["B_slots"], sched["NBATCH"], sched["TOT_PAD"], tuple(sched["Kj"]))
    if key not in _cache:
        _cache[key] = (build_launch1(sched), build_launch2(sched))
    return _cache[key]


def kernel(x, edge_index, batch, W1, b1, W2, b2, Wmu, bmu, Wls, bls,
           _trace=False):
    x = np.asarray(x, dtype=np.float32)
    cores, sched = preprocess(np.asarray(edge_index), np.asarray(batch))
    nc1, nc2 = _get_programs(sched)

    iota = np.broadcast_to(np.arange(128, dtype=np.float32), (128, 128)).astype(ml_dtypes.bfloat16)
    ident = np.eye(128, dtype=np.float32)
    iotaG = np.broadcast_to(np.arange(128, dtype=np.float32), (128, 128)).copy()
    xT = np.zeros((F, RPAD), dtype=ml_dtypes.bfloat16)
    xT[:, :N] = x.T.astype(ml_dtypes.bfloat16)
    W1 = np.asarray(W1, np.float32).astype(ml_dtypes.bfloat16)
    W2 = np.asarray(W2, np.float32).astype(ml_dtypes.bfloat16)
    Wmu = np.asarray(Wmu, np.float32); Wls = np.asarray(Wls, np.float32)
    B1 = np.broadcast_to(np.asarray(b1, np.float32), (128, H)).copy()
    B2 = np.broadcast_to(np.asarray(b2, np.float32), (128, H)).copy()
    Bmu = np.broadcast_to(np.asarray(bmu, np.float32), (128, Z)).copy()
    Bls = np.broadcast_to(np.asarray(bls, np.float32), (128, Z)).copy()

    ins1 = [dict(xT=xT, W1=W1, B1=B1, iota=iota, idx=cd["idx_w"],
                 dloc=cd["dloc_t"], cf=cd["cf_t"]) for cd in cores]
    kw = dict(trace=True) if _trace else {}
    res1 = run_bass_kernel_spmd(nc1, ins1, core_ids=list(range(NC)), **kw)

    h1 = np.zeros((N, H), dtype=np.float32)
    for c, cd in enumerate(cores):
        out = res1.results[c]["h1o"]
        for j in range(cd["nblk"]):
            b = cd["perm"][j]
            lo = cd["s"] + b * 128
            hi = min(lo + 128, cd["e"])
            h1[lo:hi] = out[j * 128 : j * 128 + (hi - lo)]

    h1T = np.zeros((H, RPAD), dtype=ml_dtypes.bfloat16)
    h1T[:, :N] = h1.T.astype(ml_dtypes.bfloat16)
    ins2 = [dict(h1T=h1T, W2=W2, B2=B2, iota=iota, idx=cd["idx_w"],
                 dloc=cd["dloc_t"], cf=cd["cf_t"], bloc=cd["bloc"],
                 iotaG=iotaG, invc=cd["invcnt"], ident=ident,
                 Wmu=Wmu, Wls=Wls, Bmu=Bmu, Bls=Bls) for cd in cores]
    res2 = run_bass_kernel_spmd(nc2, ins2, core_ids=list(range(NC)), **kw)

    mu = np.zeros((G, Z), dtype=np.float32)
    ls = np.zeros((G, Z), dtype=np.float32)
    for c, cd in enumerate(cores):
        ngr = cd["ge"] - cd["gs"]
        mu[cd["gs"] : cd["ge"]] = res2.results[c]["muo"][:ngr]
        ls[cd["gs"] : cd["ge"]] = res2.results[c]["lso"][:ngr]

    if _trace:
        kernel.last_exec_ns = (res1.exec_time_ns or 0) + (res2.exec_time_ns or 0)
        kernel.last_parts = (res1.exec_time_ns, res2.exec_time_ns)
    return mu, ls


# revision 13
# speedup vs baseline: 1.0412x; 1.0412x over previous
"""GCN encoder (2x GCNConv + mean-pool + two linear heads) on 8 NeuronCores.

Strategy (graph/data parallel, per sharding hint):
 - Nodes are range-partitioned across the 8 cores at graph boundaries
   (so global mean-pool is core-local). Each core owns the incident
   edges of its dst nodes (plus self-loops as explicit edges).
 - Layer l: every core redundantly computes the full transform table
   t = h @ W (dense matmul, cheap), then gathers t[src] rows for its own
   edges with GPSIMD dma_gather and segment-sums them per 128-node dst
   block on the TensorEngine via an on-chip-built selection matrix
   S[e, d] = coef_e * (dstlocal_e == d)   (one DVE tensor_scalar op).
 - h1 is exchanged between the two layers through the host (two NEFF
   launches; the host concatenates the 8 shards and feeds h1^T back),
   which is cheaper here than the ncfw AllGather (~110us floor).
 - Pooling + the two Z=32 projections run on-device in launch 2.

The gather index stream is int16 against a table base centered at row
32768 (signed descriptor offsets cover all 50176 rows); the last index
of each 1024-edge batch must be >= 0 (ucode trims trailing negatives),
arranged by an in-chunk swap on the host.
"""
import sys, os
sys.path.insert(0, "/opt/trn_rl_repo")
import numpy as np
import ml_dtypes

import concourse.bacc as bacc
import concourse.tile as tile
import concourse.mybir as mybir
from concourse.bass_utils import run_bass_kernel_spmd

N, F, H, Z, G = 50000, 128, 64, 32, 512
NC = 8
RPAD = 50176           # 392 * 128, padded table rows
BASE = 32768           # gather base row (signed int16 window covers [0, 65535])
CHB = 8                # chunks per gather batch (1024 edges)
NQ = 4                 # SWDGE queues (parallel Q7 descriptor generation)

_dt = mybir.dt


# ----------------------------------------------------------------- host prep

def _wrap_idx_batches(idx16):
    """[NBATCH*1024] int16 -> [NBATCH, 128, 64] wrapped+replicated layout."""
    nb = idx16.shape[0] // (CHB * 128)
    il = idx16.reshape(nb, CHB * 128)
    lanes = np.arange(CHB * 128)
    out = np.zeros((nb, 128, CHB * 128 // 16), dtype=np.int16)
    for grp in range(8):
        out[:, grp * 16 + lanes % 16, lanes // 16] = il
    return out


def preprocess(edge_index, batch):
    src = np.asarray(edge_index[0], dtype=np.int64)
    dst = np.asarray(edge_index[1], dtype=np.int64)
    batch = np.asarray(batch, dtype=np.int64)
    deg = np.bincount(dst, minlength=N).astype(np.float64) + 1.0
    dinv = 1.0 / np.sqrt(deg)
    coef = (dinv[src] * dinv[dst]).astype(np.float32)
    srcA = np.concatenate([src, np.arange(N)])
    dstA = np.concatenate([dst, np.arange(N)])
    coefA = np.concatenate([coef, (dinv * dinv).astype(np.float32)])

    gcnt = np.bincount(batch, minlength=G)
    gcum = np.concatenate([[0], np.cumsum(gcnt)])  # node index where graph g starts
    bounds = [0]
    for c in range(1, NC):
        gi = int(np.argmin(np.abs(gcum - round(c * N / NC))))
        bounds.append(int(gcum[gi]))
    bounds.append(N)
    gbounds = [int(np.searchsorted(gcum, b)) for b in bounds]  # graph index bounds

    cores = []
    for c in range(NC):
        s, e = bounds[c], bounds[c + 1]
        m = (dstA >= s) & (dstA < e)
        es, ed, ec = srcA[m], dstA[m], coefA[m]
        blk = (ed - s) // 128
        nblk = -(-(e - s) // 128)
        order = np.argsort(blk, kind="stable")
        es, ed, ec, blk = es[order], ed[order], ec[order], blk[order]
        cnt = np.bincount(blk, minlength=nblk)
        chunks = (cnt + 127) // 128
        perm = np.argsort(-chunks, kind="stable")  # block processing order
        cores.append(dict(s=s, e=e, es=es, ed=ed, ec=ec, cnt=cnt,
                          chunks=chunks, perm=perm, nblk=nblk,
                          gs=gbounds[c], ge=gbounds[c + 1]))

    B_slots = max(cd["nblk"] for cd in cores)
    Kj = np.ones(B_slots, dtype=np.int64)
    for cd in cores:
        ch = cd["chunks"][cd["perm"]]
        Kj[: cd["nblk"]] = np.maximum(Kj[: cd["nblk"]], ch)
    TOTCH = int(Kj.sum())
    TOT_PAD = -(-TOTCH // CHB) * CHB
    NBATCH = TOT_PAD // CHB
    Cj = np.concatenate([[0], np.cumsum(Kj)])  # chunk start per slot

    sched = dict(B_slots=B_slots, Kj=Kj, Cj=Cj, TOT_PAD=TOT_PAD, NBATCH=NBATCH)

    for cd in cores:
        NEdge = TOT_PAD * 128
        idx16 = np.zeros(NEdge, dtype=np.int16)
        dloc = np.full(NEdge, -1.0, dtype=np.float32)
        cf = np.zeros(NEdge, dtype=np.float32)
        estart = np.concatenate([[0], np.cumsum(cd["cnt"])])
        for j in range(cd["nblk"]):
            b = cd["perm"][j]
            k = int(cd["cnt"][b])
            if k == 0:
                continue
            e0 = int(estart[b])
            pos = Cj[j] * 128 + np.arange(k)
            idx16[pos] = (cd["es"][e0 : e0 + k] - BASE).astype(np.int16)
            dloc[pos] = (cd["ed"][e0 : e0 + k] - (cd["s"] + b * 128)).astype(np.float32)
            cf[pos] = cd["ec"][e0 : e0 + k]
        # last index of each 1024-batch must be >= 0: swap inside last chunk
        for bidx in range(NBATCH):
            last = bidx * 1024 + 1023
            if idx16[last] < 0:
                lo = bidx * 1024 + 896
                cand = np.nonzero(idx16[lo : last + 1] >= 0)[0]
                assert len(cand), "all-negative chunk: need cross-chunk swap"
                p = lo + cand[0]
                for a in (idx16, dloc, cf):
                    a[last], a[p] = a[p], a[last]
        cd["idx_w"] = _wrap_idx_batches(idx16)
        cd["dloc_t"] = dloc.reshape(TOT_PAD, 128).T.copy()
        cd["cf_t"] = cf.reshape(TOT_PAD, 128).T.copy()
        cd["idx16"] = idx16
        cd["dloc"] = dloc
        cd["cf"] = cf

        # pooling: graph-local id per (slot, lane); -1 on pads
        bloc = np.full((128, B_slots), -1.0, dtype=np.float32)
        for j in range(cd["nblk"]):
            b = cd["perm"][j]
            lo = cd["s"] + b * 128
            hi = min(lo + 128, cd["e"])
            lanes = np.arange(hi - lo)
            bloc[lanes, j] = (batch[lo:hi] - cd["gs"]).astype(np.float32)
        cd["bloc"] = bloc
        ic = np.zeros((128, 1), dtype=np.float32)
        ngr = cd["ge"] - cd["gs"]
        ic[:ngr, 0] = 1.0 / np.maximum(gcnt[cd["gs"] : cd["ge"]], 1.0)
        cd["invcnt"] = ic

    return cores, sched


# ------------------------------------------------------------- bass builders

GRP = 8  # node blocks per transform group

def _emit_table_transform(nc, tc, pools, table, hT, W_t, ntiles, kdim):
    """table rows (bf16, 128-col padded) = hT.T @ W; batched loads/writes."""
    for g0 in range(0, ntiles, GRP):
        lt = pools["xt"].tile([kdim, GRP * 128], _dt.bfloat16, tag="xt")
        nc.sync.dma_start(lt[:], hT[:kdim, g0 * 128 : (g0 + GRP) * 128])
        ct = pools["tout"].tile([128, GRP, H], _dt.bfloat16, tag="tout")
        for k in range(GRP):
            ps = pools["tps"].tile([128, H], _dt.float32, tag="tps")
            nc.tensor.matmul(ps[:], lhsT=lt[:, k * 128 : (k + 1) * 128],
                             rhs=W_t[:kdim, :], start=True, stop=True)
            nc.vector.tensor_copy(ct[:, k, :], ps[:])
        out_view = table[g0 * 128 : (g0 + GRP) * 128, 0:H].rearrange(
            "(k p) c -> p k c", k=GRP)
        nc.sync.dma_start(out_view, ct[:])


PRE_ST = 144  # selection matrices built before the table barrier

def _emit_st_prebuild(nc, pools, dloc_t, cf_t, iota_t, sched, npre=PRE_ST):
    """Build the first PRE_ST chunk selection matrices (no table dep)."""
    pre = {}
    n = min(npre, sched["TOT_PAD"])
    for cglob in range(n):
        st = pools["st"].tile([128, 128], _dt.bfloat16, tag="st")
        nc.vector.tensor_scalar(
            out=st[:], in0=iota_t[:],
            scalar1=dloc_t[:, cglob : cglob + 1],
            scalar2=cf_t[:, cglob : cglob + 1],
            op0=mybir.AluOpType.is_equal, op1=mybir.AluOpType.mult,
        )
        pre[cglob] = st
    return pre


def _emit_gather_agg(nc, tc, pools, table, idx_in, dloc_t, cf_t, iota_t, sched,
                     slot_epilogue, pre_st=None):
    """Gather stream + per-slot matmul segment-sum. slot_epilogue(j, psum)."""
    B_slots, Kj, Cj, NBATCH = sched["B_slots"], sched["Kj"], sched["Cj"], sched["NBATCH"]
    gtiles = {}
    pre_st = pre_st or {}

    def ensure_batch(b):
        if b in gtiles or b >= NBATCH:
            return
        it = pools["idx"].tile([128, CHB * 128 // 16], _dt.int16, tag="idx")
        nc.sync.dma_start(it[:], idx_in[b])
        g = pools["g"].tile([128, CHB, 128], _dt.bfloat16, tag="g")
        nc.gpsimd.dma_gather(
            out_ap=g[:], in_ap=table[BASE:, :], idxs_ap=it[:],
            num_idxs=CHB * 128, num_idxs_reg=CHB * 128, elem_size=128,
            queue_num=b % NQ,
        )
        gtiles[b] = g
        if len(gtiles) > 128:  # drop old refs (pool recycles anyway)
            del gtiles[min(gtiles)]

    ensure_batch(0)
    ensure_batch(1)
    for j in range(B_slots):
        ps = pools["aps"].tile([128, H], _dt.float32, tag="aps")
        k0, k1 = int(Cj[j]), int(Cj[j + 1])
        for cglob in range(k0, k1):
            b, col = cglob // CHB, cglob % CHB
            ensure_batch(b)
            for la in range(1, 24):
                ensure_batch(b + la)
            if cglob in pre_st:
                st = pre_st.pop(cglob)
            else:
                st = pools["st"].tile([128, 128], _dt.bfloat16, tag="st")
                nc.vector.tensor_scalar(
                    out=st[:], in0=iota_t[:],
                    scalar1=dloc_t[:, cglob : cglob + 1],
                    scalar2=cf_t[:, cglob : cglob + 1],
                    op0=mybir.AluOpType.is_equal, op1=mybir.AluOpType.mult,
                )
            nc.tensor.matmul(ps[:], lhsT=st[:], rhs=gtiles[b][:, col, 0:H],
                             start=(cglob == k0), stop=(cglob == k1 - 1))
        slot_epilogue(j, ps)


def build_launch1(sched):
    nc = bacc.Bacc("TRN2", debug=False, num_devices=NC, num_swdge_queues=NQ)
    B_slots, NBATCH, TOT_PAD = sched["B_slots"], sched["NBATCH"], sched["TOT_PAD"]

    xT = nc.dram_tensor("xT", [F, RPAD], _dt.bfloat16, kind="ExternalInput")
    W1 = nc.dram_tensor("W1", [F, H], _dt.bfloat16, kind="ExternalInput")
    B1 = nc.dram_tensor("B1", [128, H], _dt.float32, kind="ExternalInput")
    iota = nc.dram_tensor("iota", [128, 128], _dt.bfloat16, kind="ExternalInput")
    idx = nc.dram_tensor("idx", [NBATCH, 128, CHB * 128 // 16], _dt.int16, kind="ExternalInput")
    dloc = nc.dram_tensor("dloc", [128, TOT_PAD], _dt.float32, kind="ExternalInput")
    cf = nc.dram_tensor("cf", [128, TOT_PAD], _dt.float32, kind="ExternalInput")
    h1o = nc.dram_tensor("h1o", [B_slots * 128, H], _dt.float32, kind="ExternalOutput")
    t1 = nc.dram_tensor("t1", [RPAD, 128], _dt.bfloat16)

    with tile.TileContext(nc) as tc:
        with (
            tc.tile_pool(name="consts", bufs=1) as consts,
            tc.tile_pool(name="xt", bufs=3) as xt_p,
            tc.tile_pool(name="tout", bufs=4) as tout_p,
            tc.tile_pool(name="idx", bufs=32) as idx_p,
            tc.tile_pool(name="g", bufs=32) as g_p,
            tc.tile_pool(name="st", bufs=PRE_ST + 10) as st_p,
            tc.tile_pool(name="ms", bufs=10) as ms_p,
            tc.tile_pool(name="ho", bufs=4) as ho_p,
            tc.tile_pool(name="tps", bufs=2, space="PSUM") as tps_p,
            tc.tile_pool(name="aps", bufs=6, space="PSUM") as aps_p,
        ):
            pools = dict(xt=xt_p, tout=tout_p, idx=idx_p, g=g_p, st=st_p,
                         ms=ms_p, tps=tps_p, aps=aps_p)
            w1_t = consts.tile([F, H], _dt.bfloat16)
            nc.sync.dma_start(w1_t[:], W1[:, :])
            b1_t = consts.tile([128, H], _dt.float32)
            nc.sync.dma_start(b1_t[:], B1[:, :])
            iota_t = consts.tile([128, 128], _dt.bfloat16)
            nc.sync.dma_start(iota_t[:], iota[:, :])
            dloc_t = consts.tile([128, TOT_PAD], _dt.float32)
            nc.sync.dma_start(dloc_t[:], dloc[:, :])
            cf_t = consts.tile([128, TOT_PAD], _dt.float32)
            nc.sync.dma_start(cf_t[:], cf[:, :])

            pre_st = _emit_st_prebuild(nc, pools, dloc_t, cf_t, iota_t, sched)
            _emit_table_transform(nc, tc, pools, t1, xT, w1_t, RPAD // 128, F)
            tc.strict_bb_all_engine_barrier()

            def epi(j, ps):
                hb = ho_p.tile([128, H], _dt.float32, tag="ho")
                nc.vector.tensor_tensor(out=hb[:], in0=ps[:], in1=b1_t[:],
                                        op=mybir.AluOpType.add)
                nc.vector.tensor_scalar_max(hb[:], hb[:], 0.0)
                nc.sync.dma_start(h1o[j * 128 : (j + 1) * 128, :], hb[:])

            _emit_gather_agg(nc, tc, pools, t1, idx, dloc_t, cf_t, iota_t, sched, epi, pre_st)
    nc.finalize()
    return nc


def build_launch2(sched):
    nc = bacc.Bacc("TRN2", debug=False, num_devices=NC, num_swdge_queues=NQ)
    B_slots, NBATCH, TOT_PAD = sched["B_slots"], sched["NBATCH"], sched["TOT_PAD"]

    h1T = nc.dram_tensor("h1T", [H, RPAD], _dt.bfloat16, kind="ExternalInput")
    W2 = nc.dram_tensor("W2", [H, H], _dt.bfloat16, kind="ExternalInput")
    B2 = nc.dram_tensor("B2", [128, H], _dt.float32, kind="ExternalInput")
    iota = nc.dram_tensor("iota", [128, 128], _dt.bfloat16, kind="ExternalInput")
    idx = nc.dram_tensor("idx", [NBATCH, 128, CHB * 128 // 16], _dt.int16, kind="ExternalInput")
    dloc = nc.dram_tensor("dloc", [128, TOT_PAD], _dt.float32, kind="ExternalInput")
    cf = nc.dram_tensor("cf", [128, TOT_PAD], _dt.float32, kind="ExternalInput")
    bloc = nc.dram_tensor("bloc", [128, B_slots], _dt.float32, kind="ExternalInput")
    iotaG = nc.dram_tensor("iotaG", [128, 128], _dt.float32, kind="ExternalInput")
    invc = nc.dram_tensor("invc", [128, 1], _dt.float32, kind="ExternalInput")
    ident = nc.dram_tensor("ident", [128, 128], _dt.float32, kind="ExternalInput")
    Wmu = nc.dram_tensor("Wmu", [H, Z], _dt.float32, kind="ExternalInput")
    Wls = nc.dram_tensor("Wls", [H, Z], _dt.float32, kind="ExternalInput")
    Bmu = nc.dram_tensor("Bmu", [128, Z], _dt.float32, kind="ExternalInput")
    Bls = nc.dram_tensor("Bls", [128, Z], _dt.float32, kind="ExternalInput")
    muo = nc.dram_tensor("muo", [128, Z], _dt.float32, kind="ExternalOutput")
    lso = nc.dram_tensor("lso", [128, Z], _dt.float32, kind="ExternalOutput")
    t2 = nc.dram_tensor("t2", [RPAD, 128], _dt.bfloat16)

    with tile.TileContext(nc) as tc:
        with (
            tc.tile_pool(name="consts", bufs=1) as consts,
            tc.tile_pool(name="xt", bufs=3) as xt_p,
            tc.tile_pool(name="tout", bufs=4) as tout_p,
            tc.tile_pool(name="idx", bufs=32) as idx_p,
            tc.tile_pool(name="g", bufs=32) as g_p,
            tc.tile_pool(name="st", bufs=PRE_ST + 10) as st_p,
            tc.tile_pool(name="ms", bufs=10) as ms_p,
            tc.tile_pool(name="h2", bufs=4) as h2_p,
            tc.tile_pool(name="sg", bufs=4) as sg_p,
            tc.tile_pool(name="fin", bufs=8) as fin_p,
            tc.tile_pool(name="tps", bufs=2, space="PSUM") as tps_p,
            tc.tile_pool(name="aps", bufs=3, space="PSUM") as aps_p,
            tc.tile_pool(name="pps", bufs=1, space="PSUM") as pps_p,
            tc.tile_pool(name="fps", bufs=1, space="PSUM") as fps_p,
        ):
            pools = dict(xt=xt_p, tout=tout_p, idx=idx_p, g=g_p, st=st_p,
                         ms=ms_p, tps=tps_p, aps=aps_p)
            w2_t = consts.tile([H, H], _dt.bfloat16)
            nc.sync.dma_start(w2_t[:], W2[:, :])
            b2_t = consts.tile([128, H], _dt.float32)
            nc.sync.dma_start(b2_t[:], B2[:, :])
            iota_t = consts.tile([128, 128], _dt.bfloat16)
            nc.sync.dma_start(iota_t[:], iota[:, :])
            dloc_t = consts.tile([128, TOT_PAD], _dt.float32)
            nc.sync.dma_start(dloc_t[:], dloc[:, :])
            cf_t = consts.tile([128, TOT_PAD], _dt.float32)
            nc.sync.dma_start(cf_t[:], cf[:, :])
            bloc_t = consts.tile([128, B_slots], _dt.float32)
            nc.sync.dma_start(bloc_t[:], bloc[:, :])
            iog_t = consts.tile([128, 128], _dt.float32)
            nc.sync.dma_start(iog_t[:], iotaG[:, :])
            invc_t = consts.tile([128, 1], _dt.float32)
            nc.sync.dma_start(invc_t[:], invc[:, :])
            id_t = consts.tile([128, 128], _dt.float32)
            nc.sync.dma_start(id_t[:], ident[:, :])
            wmu_t = consts.tile([H, Z], _dt.float32)
            nc.sync.dma_start(wmu_t[:], Wmu[:, :])
            wls_t = consts.tile([H, Z], _dt.float32)
            nc.sync.dma_start(wls_t[:], Wls[:, :])
            bmu_t = consts.tile([128, Z], _dt.float32)
            nc.sync.dma_start(bmu_t[:], Bmu[:, :])
            bls_t = consts.tile([128, Z], _dt.float32)
            nc.sync.dma_start(bls_t[:], Bls[:, :])

            pre_st = _emit_st_prebuild(nc, pools, dloc_t, cf_t, iota_t, sched, 96)
            _emit_table_transform(nc, tc, pools, t2, h1T, w2_t, RPAD // 128, H)
            tc.strict_bb_all_engine_barrier()

            pool_ps = pps_p.tile([128, H], _dt.float32)

            def epi(j, ps):
                hb = h2_p.tile([128, H], _dt.float32, tag="h2")
                nc.vector.tensor_tensor(out=hb[:], in0=ps[:], in1=b2_t[:],
                                        op=mybir.AluOpType.add)
                nc.vector.tensor_scalar_max(hb[:], hb[:], 0.0)
                sg = sg_p.tile([128, 128], _dt.float32, tag="sg")
                nc.vector.tensor_scalar(
                    out=sg[:], in0=iog_t[:],
                    scalar1=bloc_t[:, j : j + 1], scalar2=None,
                    op0=mybir.AluOpType.is_equal,
                )
                nc.tensor.matmul(pool_ps[:], lhsT=sg[:], rhs=hb[:],
                                 start=(j == 0), stop=(j == B_slots - 1))

            _emit_gather_agg(nc, tc, pools, t2, idx, dloc_t, cf_t, iota_t, sched, epi, pre_st)

            pooled = fin_p.tile([128, H], _dt.float32, tag="pooled")
            nc.vector.tensor_scalar_mul(pooled[:], pool_ps[:], invc_t[:, 0:1])
            ptp = fps_p.tile([H, 128], _dt.float32, tag="ptp")
            nc.tensor.transpose(ptp[:], pooled[:], id_t[:])
            pooledT = fin_p.tile([H, 128], _dt.float32, tag="pooledT")
            nc.vector.tensor_copy(pooledT[:], ptp[:])
            for wt, bt, oo in ((wmu_t, bmu_t, muo), (wls_t, bls_t, lso)):
                ops = fps_p.tile([128, Z], _dt.float32, tag="ops")
                nc.tensor.matmul(ops[:], lhsT=pooledT[:], rhs=wt[:], start=True, stop=True)
                ot = fin_p.tile([128, Z], _dt.float32, tag="ot")
                nc.vector.tensor_tensor(out=ot[:], in0=ops[:], in1=bt[:],
                                        op=mybir.AluOpType.add)
                nc.sync.dma_start(oo[:, :], ot[:])
    nc.finalize()
    return nc


# ------------------------------------------------------------------- runner

_cache = {}


def _get_programs(sched):
    key = (sched["B_slots"], sched["NBATCH"], sched["TOT_PAD"], tuple(sched["Kj"]))
    if key not in _cache:
        _cache[key] = (build_launch1(sched), build_launch2(sched))
    return _cache[key]


def kernel(x, edge_index, batch, W1, b1, W2, b2, Wmu, bmu, Wls, bls,
           _trace=False):
    x = np.asarray(x, dtype=np.float32)
    cores, sched = preprocess(np.asarray(edge_index), np.asarray(batch))
    nc1, nc2 = _get_programs(sched)

    iota = np.broadcast_to(np.arange(128, dtype=np.float32), (128, 128)).astype(ml_dtypes.bfloat16)
    ident = np.eye(128, dtype=np.float32)
    iotaG = np.broadcast_to(np.arange(128, dtype=np.float32), (128, 128)).copy()
    xT = np.zeros((F, RPAD), dtype=ml_dtypes.bfloat16)
    xT[:, :N] = x.T.astype(ml_dtypes.bfloat16)
    W1 = np.asarray(W1, np.float32).astype(ml_dtypes.bfloat16)
    W2 = np.asarray(W2, np.float32).astype(ml_dtypes.bfloat16)
    Wmu = np.asarray(Wmu, np.float32); Wls = np.asarray(Wls, np.float32)
    B1 = np.broadcast_to(np.asarray(b1, np.float32), (128, H)).copy()
    B2 = np.broadcast_to(np.asarray(b2, np.float32), (128, H)).copy()
    Bmu = np.broadcast_to(np.asarray(bmu, np.float32), (128, Z)).copy()
    Bls = np.broadcast_to(np.asarray(bls, np.float32), (128, Z)).copy()

    ins1 = [dict(xT=xT, W1=W1, B1=B1, iota=iota, idx=cd["idx_w"],
                 dloc=cd["dloc_t"], cf=cd["cf_t"]) for cd in cores]
    kw = dict(trace=True) if _trace else {}
    res1 = run_bass_kernel_spmd(nc1, ins1, core_ids=list(range(NC)), **kw)

    h1 = np.zeros((N, H), dtype=np.float32)
    for c, cd in enumerate(cores):
        out = res1.results[c]["h1o"]
        for j in range(cd["nblk"]):
            b = cd["perm"][j]
            lo = cd["s"] + b * 128
            hi = min(lo + 128, cd["e"])
            h1[lo:hi] = out[j * 128 : j * 128 + (hi - lo)]

    h1T = np.zeros((H, RPAD), dtype=ml_dtypes.bfloat16)
    h1T[:, :N] = h1.T.astype(ml_dtypes.bfloat16)
    ins2 = [dict(h1T=h1T, W2=W2, B2=B2, iota=iota, idx=cd["idx_w"],
                 dloc=cd["dloc_t"], cf=cd["cf_t"], bloc=cd["bloc"],
                 iotaG=iotaG, invc=cd["invcnt"], ident=ident,
                 Wmu=Wmu, Wls=Wls, Bmu=Bmu, Bls=Bls) for cd in cores]
    res2 = run_bass_kernel_spmd(nc2, ins2, core_ids=list(range(NC)), **kw)

    mu = np.zeros((G, Z), dtype=np.float32)
    ls = np.zeros((G, Z), dtype=np.float32)
    for c, cd in enumerate(cores):
        ngr = cd["ge"] - cd["gs"]
        mu[cd["gs"] : cd["ge"]] = res2.results[c]["muo"][:ngr]
        ls[cd["gs"] : cd["ge"]] = res2.results[c]["lso"][:ngr]

    if _trace:
        kernel.last_exec_ns = (res1.exec_time_ns or 0) + (res2.exec_time_ns or 0)
        kernel.last_parts = (res1.exec_time_ns, res2.exec_time_ns)
    return mu, ls


# revision 14
# speedup vs baseline: 1.0585x; 1.0167x over previous
"""GCN encoder (2x GCNConv + mean-pool + two linear heads) on 8 NeuronCores.

Strategy (graph/data parallel, per sharding hint):
 - Nodes are range-partitioned across the 8 cores at graph boundaries
   (so global mean-pool is core-local). Each core owns the incident
   edges of its dst nodes (plus self-loops as explicit edges).
 - Layer l: every core redundantly computes the full transform table
   t = h @ W (dense matmul, cheap), then gathers t[src] rows for its own
   edges with GPSIMD dma_gather and segment-sums them per 128-node dst
   block on the TensorEngine via an on-chip-built selection matrix
   S[e, d] = coef_e * (dstlocal_e == d)   (one DVE tensor_scalar op).
 - h1 is exchanged between the two layers through the host (two NEFF
   launches; the host concatenates the 8 shards and feeds h1^T back),
   which is cheaper here than the ncfw AllGather (~110us floor).
 - Pooling + the two Z=32 projections run on-device in launch 2.

The gather index stream is int16 against a table base centered at row
32768 (signed descriptor offsets cover all 50176 rows); the last index
of each 1024-edge batch must be >= 0 (ucode trims trailing negatives),
arranged by an in-chunk swap on the host.
"""
import sys, os
sys.path.insert(0, "/opt/trn_rl_repo")
import numpy as np
import ml_dtypes

import concourse.bacc as bacc
import concourse.tile as tile
import concourse.mybir as mybir
from concourse.bass_utils import run_bass_kernel_spmd

N, F, H, Z, G = 50000, 128, 64, 32, 512
NC = 8
RPAD = 50176           # 392 * 128, padded table rows
BASE = 32768           # gather base row (signed int16 window covers [0, 65535])
CHB = 8                # chunks per gather batch (1024 edges)
NQ = 4                 # SWDGE queues (parallel Q7 descriptor generation)

_dt = mybir.dt


# ----------------------------------------------------------------- host prep

def _wrap_idx_batches(idx16):
    """[NBATCH*1024] int16 -> [NBATCH, 128, 64] wrapped+replicated layout."""
    nb = idx16.shape[0] // (CHB * 128)
    il = idx16.reshape(nb, CHB * 128)
    lanes = np.arange(CHB * 128)
    out = np.zeros((nb, 128, CHB * 128 // 16), dtype=np.int16)
    for grp in range(8):
        out[:, grp * 16 + lanes % 16, lanes // 16] = il
    return out


def preprocess(edge_index, batch):
    src = np.asarray(edge_index[0], dtype=np.int64)
    dst = np.asarray(edge_index[1], dtype=np.int64)
    batch = np.asarray(batch, dtype=np.int64)
    deg = np.bincount(dst, minlength=N).astype(np.float64) + 1.0
    dinv = 1.0 / np.sqrt(deg)
    coef = (dinv[src] * dinv[dst]).astype(np.float32)
    srcA = np.concatenate([src, np.arange(N)])
    dstA = np.concatenate([dst, np.arange(N)])
    coefA = np.concatenate([coef, (dinv * dinv).astype(np.float32)])

    gcnt = np.bincount(batch, minlength=G)
    gcum = np.concatenate([[0], np.cumsum(gcnt)])  # node index where graph g starts
    bounds = [0]
    for c in range(1, NC):
        gi = int(np.argmin(np.abs(gcum - round(c * N / NC))))
        bounds.append(int(gcum[gi]))
    bounds.append(N)
    gbounds = [int(np.searchsorted(gcum, b)) for b in bounds]  # graph index bounds

    cores = []
    for c in range(NC):
        s, e = bounds[c], bounds[c + 1]
        m = (dstA >= s) & (dstA < e)
        es, ed, ec = srcA[m], dstA[m], coefA[m]
        blk = (ed - s) // 128
        nblk = -(-(e - s) // 128)
        order = np.argsort(blk, kind="stable")
        es, ed, ec, blk = es[order], ed[order], ec[order], blk[order]
        cnt = np.bincount(blk, minlength=nblk)
        chunks = (cnt + 127) // 128
        perm = np.argsort(-chunks, kind="stable")  # block processing order
        cores.append(dict(s=s, e=e, es=es, ed=ed, ec=ec, cnt=cnt,
                          chunks=chunks, perm=perm, nblk=nblk,
                          gs=gbounds[c], ge=gbounds[c + 1]))

    B_slots = max(cd["nblk"] for cd in cores)
    Kj = np.ones(B_slots, dtype=np.int64)
    for cd in cores:
        ch = cd["chunks"][cd["perm"]]
        Kj[: cd["nblk"]] = np.maximum(Kj[: cd["nblk"]], ch)
    TOTCH = int(Kj.sum())
    TOT_PAD = -(-TOTCH // CHB) * CHB
    NBATCH = TOT_PAD // CHB
    Cj = np.concatenate([[0], np.cumsum(Kj)])  # chunk start per slot

    sched = dict(B_slots=B_slots, Kj=Kj, Cj=Cj, TOT_PAD=TOT_PAD, NBATCH=NBATCH)

    for cd in cores:
        NEdge = TOT_PAD * 128
        idx16 = np.zeros(NEdge, dtype=np.int16)
        dloc = np.full(NEdge, -1.0, dtype=np.float32)
        cf = np.zeros(NEdge, dtype=np.float32)
        estart = np.concatenate([[0], np.cumsum(cd["cnt"])])
        for j in range(cd["nblk"]):
            b = cd["perm"][j]
            k = int(cd["cnt"][b])
            if k == 0:
                continue
            e0 = int(estart[b])
            pos = Cj[j] * 128 + np.arange(k)
            idx16[pos] = (cd["es"][e0 : e0 + k] - BASE).astype(np.int16)
            dloc[pos] = (cd["ed"][e0 : e0 + k] - (cd["s"] + b * 128)).astype(np.float32)
            cf[pos] = cd["ec"][e0 : e0 + k]
        # last index of each 1024-batch must be >= 0: swap inside last chunk
        for bidx in range(NBATCH):
            last = bidx * 1024 + 1023
            if idx16[last] < 0:
                lo = bidx * 1024 + 896
                cand = np.nonzero(idx16[lo : last + 1] >= 0)[0]
                assert len(cand), "all-negative chunk: need cross-chunk swap"
                p = lo + cand[0]
                for a in (idx16, dloc, cf):
                    a[last], a[p] = a[p], a[last]
        cd["idx_w"] = _wrap_idx_batches(idx16)
        cd["dloc_t"] = dloc.reshape(TOT_PAD, 128).T.copy()
        cd["cf_t"] = cf.reshape(TOT_PAD, 128).T.copy()
        cd["idx16"] = idx16
        cd["dloc"] = dloc
        cd["cf"] = cf

        # pooling: graph-local id per (slot, lane); -1 on pads
        bloc = np.full((128, B_slots), -1.0, dtype=np.float32)
        for j in range(cd["nblk"]):
            b = cd["perm"][j]
            lo = cd["s"] + b * 128
            hi = min(lo + 128, cd["e"])
            lanes = np.arange(hi - lo)
            bloc[lanes, j] = (batch[lo:hi] - cd["gs"]).astype(np.float32)
        cd["bloc"] = bloc
        ic = np.zeros((128, 1), dtype=np.float32)
        ngr = cd["ge"] - cd["gs"]
        ic[:ngr, 0] = 1.0 / np.maximum(gcnt[cd["gs"] : cd["ge"]], 1.0)
        cd["invcnt"] = ic

    return cores, sched


# ------------------------------------------------------------- bass builders

GRP = 8  # node blocks per transform group

def _emit_table_transform(nc, tc, pools, table, hT, W_t, ntiles, kdim):
    """table rows (bf16, 128-col padded) = hT.T @ W; batched loads/writes."""
    for g0 in range(0, ntiles, GRP):
        lt = pools["xt"].tile([kdim, GRP * 128], _dt.bfloat16, tag="xt")
        nc.sync.dma_start(lt[:], hT[:kdim, g0 * 128 : (g0 + GRP) * 128])
        ct = pools["tout"].tile([128, GRP, H], _dt.bfloat16, tag="tout")
        for k in range(GRP):
            ps = pools["tps"].tile([128, H], _dt.float32, tag="tps")
            nc.tensor.matmul(ps[:], lhsT=lt[:, k * 128 : (k + 1) * 128],
                             rhs=W_t[:kdim, :], start=True, stop=True)
            nc.scalar.activation(ct[:, k, :], ps[:],
                                 mybir.ActivationFunctionType.Copy)
        out_view = table[g0 * 128 : (g0 + GRP) * 128, 0:H].rearrange(
            "(k p) c -> p k c", k=GRP)
        nc.sync.dma_start(out_view, ct[:])


PRE_ST = 144  # selection matrices built before the table barrier

def _emit_st_prebuild(nc, pools, dloc_t, cf_t, iota_t, sched, npre=PRE_ST):
    """Build the first PRE_ST chunk selection matrices (no table dep)."""
    pre = {}
    n = min(npre, sched["TOT_PAD"])
    for cglob in range(n):
        st = pools["st"].tile([128, 128], _dt.bfloat16, tag="st")
        nc.vector.tensor_scalar(
            out=st[:], in0=iota_t[:],
            scalar1=dloc_t[:, cglob : cglob + 1],
            scalar2=cf_t[:, cglob : cglob + 1],
            op0=mybir.AluOpType.is_equal, op1=mybir.AluOpType.mult,
        )
        pre[cglob] = st
    return pre


def _emit_gather_agg(nc, tc, pools, table, idx_in, dloc_t, cf_t, iota_t, sched,
                     slot_epilogue, pre_st=None):
    """Gather stream + per-slot matmul segment-sum. slot_epilogue(j, psum)."""
    B_slots, Kj, Cj, NBATCH = sched["B_slots"], sched["Kj"], sched["Cj"], sched["NBATCH"]
    gtiles = {}
    pre_st = pre_st or {}

    def ensure_batch(b):
        if b in gtiles or b >= NBATCH:
            return
        it = pools["idx"].tile([128, CHB * 128 // 16], _dt.int16, tag="idx")
        nc.sync.dma_start(it[:], idx_in[b])
        g = pools["g"].tile([128, CHB, 128], _dt.bfloat16, tag="g")
        nc.gpsimd.dma_gather(
            out_ap=g[:], in_ap=table[BASE:, :], idxs_ap=it[:],
            num_idxs=CHB * 128, num_idxs_reg=CHB * 128, elem_size=128,
            queue_num=b % NQ,
        )
        gtiles[b] = g
        if len(gtiles) > 128:  # drop old refs (pool recycles anyway)
            del gtiles[min(gtiles)]

    ensure_batch(0)
    ensure_batch(1)
    for j in range(B_slots):
        ps = pools["aps"].tile([128, H], _dt.float32, tag="aps")
        k0, k1 = int(Cj[j]), int(Cj[j + 1])
        for cglob in range(k0, k1):
            b, col = cglob // CHB, cglob % CHB
            ensure_batch(b)
            for la in range(1, 24):
                ensure_batch(b + la)
            if cglob in pre_st:
                st = pre_st.pop(cglob)
            else:
                st = pools["st"].tile([128, 128], _dt.bfloat16, tag="st")
                nc.vector.tensor_scalar(
                    out=st[:], in0=iota_t[:],
                    scalar1=dloc_t[:, cglob : cglob + 1],
                    scalar2=cf_t[:, cglob : cglob + 1],
                    op0=mybir.AluOpType.is_equal, op1=mybir.AluOpType.mult,
                )
            nc.tensor.matmul(ps[:], lhsT=st[:], rhs=gtiles[b][:, col, 0:H],
                             start=(cglob == k0), stop=(cglob == k1 - 1))
        slot_epilogue(j, ps)


def build_launch1(sched):
    nc = bacc.Bacc("TRN2", debug=False, num_devices=NC, num_swdge_queues=NQ)
    B_slots, NBATCH, TOT_PAD = sched["B_slots"], sched["NBATCH"], sched["TOT_PAD"]

    xT = nc.dram_tensor("xT", [F, RPAD], _dt.bfloat16, kind="ExternalInput")
    W1 = nc.dram_tensor("W1", [F, H], _dt.bfloat16, kind="ExternalInput")
    B1 = nc.dram_tensor("B1", [128, H], _dt.float32, kind="ExternalInput")
    iota = nc.dram_tensor("iota", [128, 128], _dt.bfloat16, kind="ExternalInput")
    idx = nc.dram_tensor("idx", [NBATCH, 128, CHB * 128 // 16], _dt.int16, kind="ExternalInput")
    dloc = nc.dram_tensor("dloc", [128, TOT_PAD], _dt.float32, kind="ExternalInput")
    cf = nc.dram_tensor("cf", [128, TOT_PAD], _dt.float32, kind="ExternalInput")
    h1o = nc.dram_tensor("h1o", [B_slots * 128, H], _dt.float32, kind="ExternalOutput")
    t1 = nc.dram_tensor("t1", [RPAD, 128], _dt.bfloat16)

    with tile.TileContext(nc) as tc:
        with (
            tc.tile_pool(name="consts", bufs=1) as consts,
            tc.tile_pool(name="xt", bufs=3) as xt_p,
            tc.tile_pool(name="tout", bufs=4) as tout_p,
            tc.tile_pool(name="idx", bufs=32) as idx_p,
            tc.tile_pool(name="g", bufs=32) as g_p,
            tc.tile_pool(name="st", bufs=PRE_ST + 10) as st_p,
            tc.tile_pool(name="ms", bufs=10) as ms_p,
            tc.tile_pool(name="ho", bufs=4) as ho_p,
            tc.tile_pool(name="tps", bufs=2, space="PSUM") as tps_p,
            tc.tile_pool(name="aps", bufs=6, space="PSUM") as aps_p,
        ):
            pools = dict(xt=xt_p, tout=tout_p, idx=idx_p, g=g_p, st=st_p,
                         ms=ms_p, tps=tps_p, aps=aps_p)
            w1_t = consts.tile([F, H], _dt.bfloat16)
            nc.sync.dma_start(w1_t[:], W1[:, :])
            b1_t = consts.tile([128, H], _dt.float32)
            nc.sync.dma_start(b1_t[:], B1[:, :])
            iota_t = consts.tile([128, 128], _dt.bfloat16)
            nc.sync.dma_start(iota_t[:], iota[:, :])
            dloc_t = consts.tile([128, TOT_PAD], _dt.float32)
            nc.sync.dma_start(dloc_t[:], dloc[:, :])
            cf_t = consts.tile([128, TOT_PAD], _dt.float32)
            nc.sync.dma_start(cf_t[:], cf[:, :])

            pre_st = _emit_st_prebuild(nc, pools, dloc_t, cf_t, iota_t, sched)
            _emit_table_transform(nc, tc, pools, t1, xT, w1_t, RPAD // 128, F)
            tc.strict_bb_all_engine_barrier()

            def epi(j, ps):
                hb = ho_p.tile([128, H], _dt.float32, tag="ho")
                nc.vector.tensor_tensor(out=hb[:], in0=ps[:], in1=b1_t[:],
                                        op=mybir.AluOpType.add)
                nc.vector.tensor_scalar_max(hb[:], hb[:], 0.0)
                nc.sync.dma_start(h1o[j * 128 : (j + 1) * 128, :], hb[:])

            _emit_gather_agg(nc, tc, pools, t1, idx, dloc_t, cf_t, iota_t, sched, epi, pre_st)
    nc.finalize()
    return nc


def build_launch2(sched):
    nc = bacc.Bacc("TRN2", debug=False, num_devices=NC, num_swdge_queues=NQ)
    B_slots, NBATCH, TOT_PAD = sched["B_slots"], sched["NBATCH"], sched["TOT_PAD"]

    h1T = nc.dram_tensor("h1T", [H, RPAD], _dt.bfloat16, kind="ExternalInput")
    W2 = nc.dram_tensor("W2", [H, H], _dt.bfloat16, kind="ExternalInput")
    B2 = nc.dram_tensor("B2", [128, H], _dt.float32, kind="ExternalInput")
    iota = nc.dram_tensor("iota", [128, 128], _dt.bfloat16, kind="ExternalInput")
    idx = nc.dram_tensor("idx", [NBATCH, 128, CHB * 128 // 16], _dt.int16, kind="ExternalInput")
    dloc = nc.dram_tensor("dloc", [128, TOT_PAD], _dt.float32, kind="ExternalInput")
    cf = nc.dram_tensor("cf", [128, TOT_PAD], _dt.float32, kind="ExternalInput")
    bloc = nc.dram_tensor("bloc", [128, B_slots], _dt.float32, kind="ExternalInput")
    iotaG = nc.dram_tensor("iotaG", [128, 128], _dt.float32, kind="ExternalInput")
    invc = nc.dram_tensor("invc", [128, 1], _dt.float32, kind="ExternalInput")
    ident = nc.dram_tensor("ident", [128, 128], _dt.float32, kind="ExternalInput")
    Wmu = nc.dram_tensor("Wmu", [H, Z], _dt.float32, kind="ExternalInput")
    Wls = nc.dram_tensor("Wls", [H, Z], _dt.float32, kind="ExternalInput")
    Bmu = nc.dram_tensor("Bmu", [128, Z], _dt.float32, kind="ExternalInput")
    Bls = nc.dram_tensor("Bls", [128, Z], _dt.float32, kind="ExternalInput")
    muo = nc.dram_tensor("muo", [128, Z], _dt.float32, kind="ExternalOutput")
    lso = nc.dram_tensor("lso", [128, Z], _dt.float32, kind="ExternalOutput")
    t2 = nc.dram_tensor("t2", [RPAD, 128], _dt.bfloat16)

    with tile.TileContext(nc) as tc:
        with (
            tc.tile_pool(name="consts", bufs=1) as consts,
            tc.tile_pool(name="xt", bufs=3) as xt_p,
            tc.tile_pool(name="tout", bufs=4) as tout_p,
            tc.tile_pool(name="idx", bufs=32) as idx_p,
            tc.tile_pool(name="g", bufs=32) as g_p,
            tc.tile_pool(name="st", bufs=PRE_ST + 10) as st_p,
            tc.tile_pool(name="ms", bufs=10) as ms_p,
            tc.tile_pool(name="h2", bufs=4) as h2_p,
            tc.tile_pool(name="sg", bufs=4) as sg_p,
            tc.tile_pool(name="fin", bufs=8) as fin_p,
            tc.tile_pool(name="tps", bufs=2, space="PSUM") as tps_p,
            tc.tile_pool(name="aps", bufs=3, space="PSUM") as aps_p,
            tc.tile_pool(name="pps", bufs=1, space="PSUM") as pps_p,
            tc.tile_pool(name="fps", bufs=1, space="PSUM") as fps_p,
        ):
            pools = dict(xt=xt_p, tout=tout_p, idx=idx_p, g=g_p, st=st_p,
                         ms=ms_p, tps=tps_p, aps=aps_p)
            w2_t = consts.tile([H, H], _dt.bfloat16)
            nc.sync.dma_start(w2_t[:], W2[:, :])
            b2_t = consts.tile([128, H], _dt.float32)
            nc.sync.dma_start(b2_t[:], B2[:, :])
            iota_t = consts.tile([128, 128], _dt.bfloat16)
            nc.sync.dma_start(iota_t[:], iota[:, :])
            dloc_t = consts.tile([128, TOT_PAD], _dt.float32)
            nc.sync.dma_start(dloc_t[:], dloc[:, :])
            cf_t = consts.tile([128, TOT_PAD], _dt.float32)
            nc.sync.dma_start(cf_t[:], cf[:, :])
            bloc_t = consts.tile([128, B_slots], _dt.float32)
            nc.sync.dma_start(bloc_t[:], bloc[:, :])
            iog_t = consts.tile([128, 128], _dt.float32)
            nc.sync.dma_start(iog_t[:], iotaG[:, :])
            invc_t = consts.tile([128, 1], _dt.float32)
            nc.sync.dma_start(invc_t[:], invc[:, :])
            id_t = consts.tile([128, 128], _dt.float32)
            nc.sync.dma_start(id_t[:], ident[:, :])
            wmu_t = consts.tile([H, Z], _dt.float32)
            nc.sync.dma_start(wmu_t[:], Wmu[:, :])
            wls_t = consts.tile([H, Z], _dt.float32)
            nc.sync.dma_start(wls_t[:], Wls[:, :])
            bmu_t = consts.tile([128, Z], _dt.float32)
            nc.sync.dma_start(bmu_t[:], Bmu[:, :])
            bls_t = consts.tile([128, Z], _dt.float32)
            nc.sync.dma_start(bls_t[:], Bls[:, :])

            pre_st = _emit_st_prebuild(nc, pools, dloc_t, cf_t, iota_t, sched, 96)
            _emit_table_transform(nc, tc, pools, t2, h1T, w2_t, RPAD // 128, H)
            tc.strict_bb_all_engine_barrier()

            pool_ps = pps_p.tile([128, H], _dt.float32)

            def epi(j, ps):
                hb = h2_p.tile([128, H], _dt.float32, tag="h2")
                nc.vector.tensor_tensor(out=hb[:], in0=ps[:], in1=b2_t[:],
                                        op=mybir.AluOpType.add)
                nc.vector.tensor_scalar_max(hb[:], hb[:], 0.0)
                sg = sg_p.tile([128, 128], _dt.float32, tag="sg")
                nc.vector.tensor_scalar(
                    out=sg[:], in0=iog_t[:],
                    scalar1=bloc_t[:, j : j + 1], scalar2=None,
                    op0=mybir.AluOpType.is_equal,
                )
                nc.tensor.matmul(pool_ps[:], lhsT=sg[:], rhs=hb[:],
                                 start=(j == 0), stop=(j == B_slots - 1))

            _emit_gather_agg(nc, tc, pools, t2, idx, dloc_t, cf_t, iota_t, sched, epi, pre_st)

            pooled = fin_p.tile([128, H], _dt.float32, tag="pooled")
            nc.vector.tensor_scalar_mul(pooled[:], pool_ps[:], invc_t[:, 0:1])
            ptp = fps_p.tile([H, 128], _dt.float32, tag="ptp")
            nc.tensor.transpose(ptp[:], pooled[:], id_t[:])
            pooledT = fin_p.tile([H, 128], _dt.float32, tag="pooledT")
            nc.vector.tensor_copy(pooledT[:], ptp[:])
            for wt, bt, oo in ((wmu_t, bmu_t, muo), (wls_t, bls_t, lso)):
                ops = fps_p.tile([128, Z], _dt.float32, tag="ops")
                nc.tensor.matmul(ops[:], lhsT=pooledT[:], rhs=wt[:], start=True, stop=True)
                ot = fin_p.tile([128, Z], _dt.float32, tag="ot")
                nc.vector.tensor_tensor(out=ot[:], in0=ops[:], in1=bt[:],
                                        op=mybir.AluOpType.add)
                nc.sync.dma_start(oo[:, :], ot[:])
    nc.finalize()
    return nc


# ------------------------------------------------------------------- runner

_cache = {}


def _get_programs(sched):
    key = (sched["B_slots"], sched["NBATCH"], sched["TOT_PAD"], tuple(sched["Kj"]))
    if key not in _cache:
        _cache[key] = (build_launch1(sched), build_launch2(sched))
    return _cache[key]


def kernel(x, edge_index, batch, W1, b1, W2, b2, Wmu, bmu, Wls, bls,
           _trace=False):
    x = np.asarray(x, dtype=np.float32)
    cores, sched = preprocess(np.asarray(edge_index), np.asarray(batch))
    nc1, nc2 = _get_programs(sched)

    iota = np.broadcast_to(np.arange(128, dtype=np.float32), (128, 128)).astype(ml_dtypes.bfloat16)
    ident = np.eye(128, dtype=np.float32)
    iotaG = np.broadcast_to(np.arange(128, dtype=np.float32), (128, 128)).copy()
    xT = np.zeros((F, RPAD), dtype=ml_dtypes.bfloat16)
    xT[:, :N] = x.T.astype(ml_dtypes.bfloat16)
    W1 = np.asarray(W1, np.float32).astype(ml_dtypes.bfloat16)
    W2 = np.asarray(W2, np.float32).astype(ml_dtypes.bfloat16)
    Wmu = np.asarray(Wmu, np.float32); Wls = np.asarray(Wls, np.float32)
    B1 = np.broadcast_to(np.asarray(b1, np.float32), (128, H)).copy()
    B2 = np.broadcast_to(np.asarray(b2, np.float32), (128, H)).copy()
    Bmu = np.broadcast_to(np.asarray(bmu, np.float32), (128, Z)).copy()
    Bls = np.broadcast_to(np.asarray(bls, np.float32), (128, Z)).copy()

    ins1 = [dict(xT=xT, W1=W1, B1=B1, iota=iota, idx=cd["idx_w"],
                 dloc=cd["dloc_t"], cf=cd["cf_t"]) for cd in cores]
    kw = dict(trace=True) if _trace else {}
    res1 = run_bass_kernel_spmd(nc1, ins1, core_ids=list(range(NC)), **kw)

    h1 = np.zeros((N, H), dtype=np.float32)
    for c, cd in enumerate(cores):
        out = res1.results[c]["h1o"]
        for j in range(cd["nblk"]):
            b = cd["perm"][j]
            lo = cd["s"] + b * 128
            hi = min(lo + 128, cd["e"])
            h1[lo:hi] = out[j * 128 : j * 128 + (hi - lo)]

    h1T = np.zeros((H, RPAD), dtype=ml_dtypes.bfloat16)
    h1T[:, :N] = h1.T.astype(ml_dtypes.bfloat16)
    ins2 = [dict(h1T=h1T, W2=W2, B2=B2, iota=iota, idx=cd["idx_w"],
                 dloc=cd["dloc_t"], cf=cd["cf_t"], bloc=cd["bloc"],
                 iotaG=iotaG, invc=cd["invcnt"], ident=ident,
                 Wmu=Wmu, Wls=Wls, Bmu=Bmu, Bls=Bls) for cd in cores]
    res2 = run_bass_kernel_spmd(nc2, ins2, core_ids=list(range(NC)), **kw)

    mu = np.zeros((G, Z), dtype=np.float32)
    ls = np.zeros((G, Z), dtype=np.float32)
    for c, cd in enumerate(cores):
        ngr = cd["ge"] - cd["gs"]
        mu[cd["gs"] : cd["ge"]] = res2.results[c]["muo"][:ngr]
        ls[cd["gs"] : cd["ge"]] = res2.results[c]["lso"][:ngr]

    if _trace:
        kernel.last_exec_ns = (res1.exec_time_ns or 0) + (res2.exec_time_ns or 0)
        kernel.last_parts = (res1.exec_time_ns, res2.exec_time_ns)
    return mu, ls


# revision 15
# speedup vs baseline: 1.0903x; 1.0300x over previous
"""GCN encoder (2x GCNConv + mean-pool + two linear heads) on 8 NeuronCores.

Strategy (graph/data parallel, per sharding hint):
 - Nodes are range-partitioned across the 8 cores at graph boundaries
   (so global mean-pool is core-local). Each core owns the incident
   edges of its dst nodes (plus self-loops as explicit edges).
 - Layer l: every core redundantly computes the full transform table
   t = h @ W (dense matmul, cheap), then gathers t[src] rows for its own
   edges with GPSIMD dma_gather and segment-sums them per 128-node dst
   block on the TensorEngine via an on-chip-built selection matrix
   S[e, d] = coef_e * (dstlocal_e == d)   (one DVE tensor_scalar op).
 - h1 is exchanged between the two layers through the host (two NEFF
   launches; the host concatenates the 8 shards and feeds h1^T back),
   which is cheaper here than the ncfw AllGather (~110us floor).
 - Pooling + the two Z=32 projections run on-device in launch 2.

The gather index stream is int16 against a table base centered at row
32768 (signed descriptor offsets cover all 50176 rows); the last index
of each 1024-edge batch must be >= 0 (ucode trims trailing negatives),
arranged by an in-chunk swap on the host.
"""
import sys, os
sys.path.insert(0, "/opt/trn_rl_repo")
import numpy as np
import ml_dtypes

import concourse.bacc as bacc
import concourse.tile as tile
import concourse.mybir as mybir
from concourse.bass_utils import run_bass_kernel_spmd

N, F, H, Z, G = 50000, 128, 64, 32, 512
NC = 8
RPAD = 50176           # 392 * 128, padded table rows
BASE = 32768           # gather base row (signed int16 window covers [0, 65535])
CHB = 8                # chunks per gather batch (1024 edges)
NQ = 4                 # SWDGE queues (parallel Q7 descriptor generation)

_dt = mybir.dt


# ----------------------------------------------------------------- host prep

def _wrap_idx_batches(idx16):
    """[NBATCH*1024] int16 -> [NBATCH, 128, 64] wrapped+replicated layout."""
    nb = idx16.shape[0] // (CHB * 128)
    il = idx16.reshape(nb, CHB * 128)
    lanes = np.arange(CHB * 128)
    out = np.zeros((nb, 128, CHB * 128 // 16), dtype=np.int16)
    for grp in range(8):
        out[:, grp * 16 + lanes % 16, lanes // 16] = il
    return out


def preprocess(edge_index, batch):
    src = np.asarray(edge_index[0], dtype=np.int64)
    dst = np.asarray(edge_index[1], dtype=np.int64)
    batch = np.asarray(batch, dtype=np.int64)
    deg = np.bincount(dst, minlength=N).astype(np.float64) + 1.0
    dinv = 1.0 / np.sqrt(deg)
    coef = (dinv[src] * dinv[dst]).astype(np.float32)
    srcA = np.concatenate([src, np.arange(N)])
    dstA = np.concatenate([dst, np.arange(N)])
    coefA = np.concatenate([coef, (dinv * dinv).astype(np.float32)])

    gcnt = np.bincount(batch, minlength=G)
    gcum = np.concatenate([[0], np.cumsum(gcnt)])  # node index where graph g starts
    bounds = [0]
    for c in range(1, NC):
        gi = int(np.argmin(np.abs(gcum - round(c * N / NC))))
        bounds.append(int(gcum[gi]))
    bounds.append(N)
    gbounds = [int(np.searchsorted(gcum, b)) for b in bounds]  # graph index bounds

    cores = []
    for c in range(NC):
        s, e = bounds[c], bounds[c + 1]
        m = (dstA >= s) & (dstA < e)
        es, ed, ec = srcA[m], dstA[m], coefA[m]
        blk = (ed - s) // 128
        nblk = -(-(e - s) // 128)
        order = np.argsort(blk, kind="stable")
        es, ed, ec, blk = es[order], ed[order], ec[order], blk[order]
        cnt = np.bincount(blk, minlength=nblk)
        chunks = (cnt + 127) // 128
        perm = np.argsort(-chunks, kind="stable")  # block processing order
        cores.append(dict(s=s, e=e, es=es, ed=ed, ec=ec, cnt=cnt,
                          chunks=chunks, perm=perm, nblk=nblk,
                          gs=gbounds[c], ge=gbounds[c + 1]))

    B_slots = max(cd["nblk"] for cd in cores)
    Kj = np.ones(B_slots, dtype=np.int64)
    for cd in cores:
        ch = cd["chunks"][cd["perm"]]
        Kj[: cd["nblk"]] = np.maximum(Kj[: cd["nblk"]], ch)
    TOTCH = int(Kj.sum())
    TOT_PAD = -(-TOTCH // CHB) * CHB
    NBATCH = TOT_PAD // CHB
    Cj = np.concatenate([[0], np.cumsum(Kj)])  # chunk start per slot

    sched = dict(B_slots=B_slots, Kj=Kj, Cj=Cj, TOT_PAD=TOT_PAD, NBATCH=NBATCH)

    for cd in cores:
        NEdge = TOT_PAD * 128
        idx16 = np.zeros(NEdge, dtype=np.int16)
        dloc = np.full(NEdge, -1.0, dtype=np.float32)
        cf = np.zeros(NEdge, dtype=np.float32)
        estart = np.concatenate([[0], np.cumsum(cd["cnt"])])
        for j in range(cd["nblk"]):
            b = cd["perm"][j]
            k = int(cd["cnt"][b])
            if k == 0:
                continue
            e0 = int(estart[b])
            pos = Cj[j] * 128 + np.arange(k)
            idx16[pos] = (cd["es"][e0 : e0 + k] - BASE).astype(np.int16)
            dloc[pos] = (cd["ed"][e0 : e0 + k] - (cd["s"] + b * 128)).astype(np.float32)
            cf[pos] = cd["ec"][e0 : e0 + k]
        # last index of each 1024-batch must be >= 0: swap inside last chunk
        for bidx in range(NBATCH):
            last = bidx * 1024 + 1023
            if idx16[last] < 0:
                lo = bidx * 1024 + 896
                cand = np.nonzero(idx16[lo : last + 1] >= 0)[0]
                assert len(cand), "all-negative chunk: need cross-chunk swap"
                p = lo + cand[0]
                for a in (idx16, dloc, cf):
                    a[last], a[p] = a[p], a[last]
        cd["idx_w"] = _wrap_idx_batches(idx16)
        cd["dloc_t"] = dloc.reshape(TOT_PAD, 128).T.copy()
        cd["cf_t"] = cf.reshape(TOT_PAD, 128).T.copy()
        cd["idx16"] = idx16
        cd["dloc"] = dloc
        cd["cf"] = cf

        # pooling: graph-local id per (slot, lane); -1 on pads
        bloc = np.full((128, B_slots), -1.0, dtype=np.float32)
        for j in range(cd["nblk"]):
            b = cd["perm"][j]
            lo = cd["s"] + b * 128
            hi = min(lo + 128, cd["e"])
            lanes = np.arange(hi - lo)
            bloc[lanes, j] = (batch[lo:hi] - cd["gs"]).astype(np.float32)
        cd["bloc"] = bloc
        ic = np.zeros((128, 1), dtype=np.float32)
        ngr = cd["ge"] - cd["gs"]
        ic[:ngr, 0] = 1.0 / np.maximum(gcnt[cd["gs"] : cd["ge"]], 1.0)
        cd["invcnt"] = ic

    return cores, sched


# ------------------------------------------------------------- bass builders

GRP = 14  # node blocks per transform group (392 = 28*14)

def _emit_table_transform(nc, tc, pools, table, hT, W_t, ntiles, kdim):
    """table rows (bf16, 128-col padded) = hT.T @ W; batched loads/writes."""
    for g0 in range(0, ntiles, GRP):
        lt = pools["xt"].tile([kdim, GRP * 128], _dt.bfloat16, tag="xt")
        nc.sync.dma_start(lt[:], hT[:kdim, g0 * 128 : (g0 + GRP) * 128])
        ct = pools["tout"].tile([128, GRP, H], _dt.bfloat16, tag="tout")
        for k in range(GRP):
            ps = pools["tps"].tile([128, H], _dt.float32, tag="tps")
            nc.tensor.matmul(ps[:], lhsT=lt[:, k * 128 : (k + 1) * 128],
                             rhs=W_t[:kdim, :], start=True, stop=True)
            nc.scalar.activation(ct[:, k, :], ps[:],
                                 mybir.ActivationFunctionType.Copy)
        out_view = table[g0 * 128 : (g0 + GRP) * 128, 0:H].rearrange(
            "(k p) c -> p k c", k=GRP)
        nc.sync.dma_start(out_view, ct[:])


PRE_ST = 144  # selection matrices built before the table barrier

def _emit_st_prebuild(nc, pools, dloc_t, cf_t, iota_t, sched, npre=PRE_ST):
    """Build the first PRE_ST chunk selection matrices (no table dep)."""
    pre = {}
    n = min(npre, sched["TOT_PAD"])
    for cglob in range(n):
        st = pools["st"].tile([128, 128], _dt.bfloat16, tag="st")
        nc.vector.tensor_scalar(
            out=st[:], in0=iota_t[:],
            scalar1=dloc_t[:, cglob : cglob + 1],
            scalar2=cf_t[:, cglob : cglob + 1],
            op0=mybir.AluOpType.is_equal, op1=mybir.AluOpType.mult,
        )
        pre[cglob] = st
    return pre


def _emit_gather_agg(nc, tc, pools, table, idx_in, dloc_t, cf_t, iota_t, sched,
                     slot_epilogue, pre_st=None):
    """Gather stream + per-slot matmul segment-sum. slot_epilogue(j, psum)."""
    B_slots, Kj, Cj, NBATCH = sched["B_slots"], sched["Kj"], sched["Cj"], sched["NBATCH"]
    gtiles = {}
    pre_st = pre_st or {}

    def ensure_batch(b):
        if b in gtiles or b >= NBATCH:
            return
        it = pools["idx"].tile([128, CHB * 128 // 16], _dt.int16, tag="idx")
        nc.sync.dma_start(it[:], idx_in[b])
        g = pools["g"].tile([128, CHB, 128], _dt.bfloat16, tag="g")
        nc.gpsimd.dma_gather(
            out_ap=g[:], in_ap=table[BASE:, :], idxs_ap=it[:],
            num_idxs=CHB * 128, num_idxs_reg=CHB * 128, elem_size=128,
            queue_num=b % NQ,
        )
        gtiles[b] = g
        if len(gtiles) > 128:  # drop old refs (pool recycles anyway)
            del gtiles[min(gtiles)]

    ensure_batch(0)
    ensure_batch(1)
    for j in range(B_slots):
        ps = pools["aps"].tile([128, H], _dt.float32, tag="aps")
        k0, k1 = int(Cj[j]), int(Cj[j + 1])
        for cglob in range(k0, k1):
            b, col = cglob // CHB, cglob % CHB
            ensure_batch(b)
            for la in range(1, 30):
                ensure_batch(b + la)
            if cglob in pre_st:
                st = pre_st.pop(cglob)
            else:
                st = pools["st"].tile([128, 128], _dt.bfloat16, tag="st")
                nc.vector.tensor_scalar(
                    out=st[:], in0=iota_t[:],
                    scalar1=dloc_t[:, cglob : cglob + 1],
                    scalar2=cf_t[:, cglob : cglob + 1],
                    op0=mybir.AluOpType.is_equal, op1=mybir.AluOpType.mult,
                )
            nc.tensor.matmul(ps[:], lhsT=st[:], rhs=gtiles[b][:, col, 0:H],
                             start=(cglob == k0), stop=(cglob == k1 - 1))
        slot_epilogue(j, ps)


def build_launch1(sched):
    nc = bacc.Bacc("TRN2", debug=False, num_devices=NC, num_swdge_queues=NQ)
    B_slots, NBATCH, TOT_PAD = sched["B_slots"], sched["NBATCH"], sched["TOT_PAD"]

    xT = nc.dram_tensor("xT", [F, RPAD], _dt.bfloat16, kind="ExternalInput")
    W1 = nc.dram_tensor("W1", [F, H], _dt.bfloat16, kind="ExternalInput")
    B1 = nc.dram_tensor("B1", [128, H], _dt.float32, kind="ExternalInput")
    iota = nc.dram_tensor("iota", [128, 128], _dt.bfloat16, kind="ExternalInput")
    idx = nc.dram_tensor("idx", [NBATCH, 128, CHB * 128 // 16], _dt.int16, kind="ExternalInput")
    dloc = nc.dram_tensor("dloc", [128, TOT_PAD], _dt.float32, kind="ExternalInput")
    cf = nc.dram_tensor("cf", [128, TOT_PAD], _dt.float32, kind="ExternalInput")
    h1o = nc.dram_tensor("h1o", [B_slots * 128, H], _dt.float32, kind="ExternalOutput")
    t1 = nc.dram_tensor("t1", [RPAD, 128], _dt.bfloat16)

    with tile.TileContext(nc) as tc:
        with (
            tc.tile_pool(name="consts", bufs=1) as consts,
            tc.tile_pool(name="xt", bufs=3) as xt_p,
            tc.tile_pool(name="tout", bufs=4) as tout_p,
            tc.tile_pool(name="idx", bufs=32) as idx_p,
            tc.tile_pool(name="g", bufs=32) as g_p,
            tc.tile_pool(name="st", bufs=PRE_ST + 10) as st_p,
            tc.tile_pool(name="ms", bufs=10) as ms_p,
            tc.tile_pool(name="ho", bufs=4) as ho_p,
            tc.tile_pool(name="tps", bufs=2, space="PSUM") as tps_p,
            tc.tile_pool(name="aps", bufs=6, space="PSUM") as aps_p,
        ):
            pools = dict(xt=xt_p, tout=tout_p, idx=idx_p, g=g_p, st=st_p,
                         ms=ms_p, tps=tps_p, aps=aps_p)
            w1_t = consts.tile([F, H], _dt.bfloat16)
            nc.sync.dma_start(w1_t[:], W1[:, :])
            b1_t = consts.tile([128, H], _dt.float32)
            nc.sync.dma_start(b1_t[:], B1[:, :])
            iota_t = consts.tile([128, 128], _dt.bfloat16)
            nc.sync.dma_start(iota_t[:], iota[:, :])
            dloc_t = consts.tile([128, TOT_PAD], _dt.float32)
            nc.sync.dma_start(dloc_t[:], dloc[:, :])
            cf_t = consts.tile([128, TOT_PAD], _dt.float32)
            nc.sync.dma_start(cf_t[:], cf[:, :])

            pre_st = _emit_st_prebuild(nc, pools, dloc_t, cf_t, iota_t, sched)
            _emit_table_transform(nc, tc, pools, t1, xT, w1_t, RPAD // 128, F)
            tc.strict_bb_all_engine_barrier()

            def epi(j, ps):
                hb = ho_p.tile([128, H], _dt.float32, tag="ho")
                nc.vector.tensor_tensor(out=hb[:], in0=ps[:], in1=b1_t[:],
                                        op=mybir.AluOpType.add)
                hb2 = ho_p.tile([128, H], _dt.float32, tag="ho2")
                nc.scalar.activation(hb2[:], hb[:],
                                     mybir.ActivationFunctionType.Relu)
                hb = hb2
                nc.sync.dma_start(h1o[j * 128 : (j + 1) * 128, :], hb[:])

            _emit_gather_agg(nc, tc, pools, t1, idx, dloc_t, cf_t, iota_t, sched, epi, pre_st)
    nc.finalize()
    return nc


def build_launch2(sched):
    nc = bacc.Bacc("TRN2", debug=False, num_devices=NC, num_swdge_queues=NQ)
    B_slots, NBATCH, TOT_PAD = sched["B_slots"], sched["NBATCH"], sched["TOT_PAD"]

    h1T = nc.dram_tensor("h1T", [H, RPAD], _dt.bfloat16, kind="ExternalInput")
    W2 = nc.dram_tensor("W2", [H, H], _dt.bfloat16, kind="ExternalInput")
    B2 = nc.dram_tensor("B2", [128, H], _dt.float32, kind="ExternalInput")
    iota = nc.dram_tensor("iota", [128, 128], _dt.bfloat16, kind="ExternalInput")
    idx = nc.dram_tensor("idx", [NBATCH, 128, CHB * 128 // 16], _dt.int16, kind="ExternalInput")
    dloc = nc.dram_tensor("dloc", [128, TOT_PAD], _dt.float32, kind="ExternalInput")
    cf = nc.dram_tensor("cf", [128, TOT_PAD], _dt.float32, kind="ExternalInput")
    bloc = nc.dram_tensor("bloc", [128, B_slots], _dt.float32, kind="ExternalInput")
    iotaG = nc.dram_tensor("iotaG", [128, 128], _dt.float32, kind="ExternalInput")
    invc = nc.dram_tensor("invc", [128, 1], _dt.float32, kind="ExternalInput")
    ident = nc.dram_tensor("ident", [128, 128], _dt.float32, kind="ExternalInput")
    Wmu = nc.dram_tensor("Wmu", [H, Z], _dt.float32, kind="ExternalInput")
    Wls = nc.dram_tensor("Wls", [H, Z], _dt.float32, kind="ExternalInput")
    Bmu = nc.dram_tensor("Bmu", [128, Z], _dt.float32, kind="ExternalInput")
    Bls = nc.dram_tensor("Bls", [128, Z], _dt.float32, kind="ExternalInput")
    muo = nc.dram_tensor("muo", [128, Z], _dt.float32, kind="ExternalOutput")
    lso = nc.dram_tensor("lso", [128, Z], _dt.float32, kind="ExternalOutput")
    t2 = nc.dram_tensor("t2", [RPAD, 128], _dt.bfloat16)

    with tile.TileContext(nc) as tc:
        with (
            tc.tile_pool(name="consts", bufs=1) as consts,
            tc.tile_pool(name="xt", bufs=3) as xt_p,
            tc.tile_pool(name="tout", bufs=4) as tout_p,
            tc.tile_pool(name="idx", bufs=32) as idx_p,
            tc.tile_pool(name="g", bufs=32) as g_p,
            tc.tile_pool(name="st", bufs=PRE_ST + 10) as st_p,
            tc.tile_pool(name="ms", bufs=10) as ms_p,
            tc.tile_pool(name="h2", bufs=4) as h2_p,
            tc.tile_pool(name="sg", bufs=4) as sg_p,
            tc.tile_pool(name="fin", bufs=8) as fin_p,
            tc.tile_pool(name="tps", bufs=2, space="PSUM") as tps_p,
            tc.tile_pool(name="aps", bufs=3, space="PSUM") as aps_p,
            tc.tile_pool(name="pps", bufs=1, space="PSUM") as pps_p,
            tc.tile_pool(name="fps", bufs=1, space="PSUM") as fps_p,
        ):
            pools = dict(xt=xt_p, tout=tout_p, idx=idx_p, g=g_p, st=st_p,
                         ms=ms_p, tps=tps_p, aps=aps_p)
            w2_t = consts.tile([H, H], _dt.bfloat16)
            nc.sync.dma_start(w2_t[:], W2[:, :])
            b2_t = consts.tile([128, H], _dt.float32)
            nc.sync.dma_start(b2_t[:], B2[:, :])
            iota_t = consts.tile([128, 128], _dt.bfloat16)
            nc.sync.dma_start(iota_t[:], iota[:, :])
            dloc_t = consts.tile([128, TOT_PAD], _dt.float32)
            nc.sync.dma_start(dloc_t[:], dloc[:, :])
            cf_t = consts.tile([128, TOT_PAD], _dt.float32)
            nc.sync.dma_start(cf_t[:], cf[:, :])
            bloc_t = consts.tile([128, B_slots], _dt.float32)
            nc.sync.dma_start(bloc_t[:], bloc[:, :])
            iog_t = consts.tile([128, 128], _dt.float32)
            nc.sync.dma_start(iog_t[:], iotaG[:, :])
            invc_t = consts.tile([128, 1], _dt.float32)
            nc.sync.dma_start(invc_t[:], invc[:, :])
            id_t = consts.tile([128, 128], _dt.float32)
            nc.sync.dma_start(id_t[:], ident[:, :])
            wmu_t = consts.tile([H, Z], _dt.float32)
            nc.sync.dma_start(wmu_t[:], Wmu[:, :])
            wls_t = consts.tile([H, Z], _dt.float32)
            nc.sync.dma_start(wls_t[:], Wls[:, :])
            bmu_t = consts.tile([128, Z], _dt.float32)
            nc.sync.dma_start(bmu_t[:], Bmu[:, :])
            bls_t = consts.tile([128, Z], _dt.float32)
            nc.sync.dma_start(bls_t[:], Bls[:, :])

            pre_st = _emit_st_prebuild(nc, pools, dloc_t, cf_t, iota_t, sched, 96)
            _emit_table_transform(nc, tc, pools, t2, h1T, w2_t, RPAD // 128, H)
            tc.strict_bb_all_engine_barrier()

            pool_ps = pps_p.tile([128, H], _dt.float32)

            def epi(j, ps):
                hb = h2_p.tile([128, H], _dt.float32, tag="h2")
                nc.vector.tensor_tensor(out=hb[:], in0=ps[:], in1=b2_t[:],
                                        op=mybir.AluOpType.add)
                hb2 = h2_p.tile([128, H], _dt.float32, tag="h22")
                nc.scalar.activation(hb2[:], hb[:],
                                     mybir.ActivationFunctionType.Relu)
                hb = hb2
                sg = sg_p.tile([128, 128], _dt.float32, tag="sg")
                nc.vector.tensor_scalar(
                    out=sg[:], in0=iog_t[:],
                    scalar1=bloc_t[:, j : j + 1], scalar2=None,
                    op0=mybir.AluOpType.is_equal,
                )
                nc.tensor.matmul(pool_ps[:], lhsT=sg[:], rhs=hb[:],
                                 start=(j == 0), stop=(j == B_slots - 1))

            _emit_gather_agg(nc, tc, pools, t2, idx, dloc_t, cf_t, iota_t, sched, epi, pre_st)

            pooled = fin_p.tile([128, H], _dt.float32, tag="pooled")
            nc.vector.tensor_scalar_mul(pooled[:], pool_ps[:], invc_t[:, 0:1])
            ptp = fps_p.tile([H, 128], _dt.float32, tag="ptp")
            nc.tensor.transpose(ptp[:], pooled[:], id_t[:])
            pooledT = fin_p.tile([H, 128], _dt.float32, tag="pooledT")
            nc.vector.tensor_copy(pooledT[:], ptp[:])
            for wt, bt, oo in ((wmu_t, bmu_t, muo), (wls_t, bls_t, lso)):
                ops = fps_p.tile([128, Z], _dt.float32, tag="ops")
                nc.tensor.matmul(ops[:], lhsT=pooledT[:], rhs=wt[:], start=True, stop=True)
                ot = fin_p.tile([128, Z], _dt.float32, tag="ot")
                nc.vector.tensor_tensor(out=ot[:], in0=ops[:], in1=bt[:],
                                        op=mybir.AluOpType.add)
                nc.sync.dma_start(oo[:, :], ot[:])
    nc.finalize()
    return nc


# ------------------------------------------------------------------- runner

_cache = {}


def _get_programs(sched):
    key = (sched["B_slots"], sched["NBATCH"], sched["TOT_PAD"], tuple(sched["Kj"]))
    if key not in _cache:
        _cache[key] = (build_launch1(sched), build_launch2(sched))
    return _cache[key]


def kernel(x, edge_index, batch, W1, b1, W2, b2, Wmu, bmu, Wls, bls,
           _trace=False):
    x = np.asarray(x, dtype=np.float32)
    cores, sched = preprocess(np.asarray(edge_index), np.asarray(batch))
    nc1, nc2 = _get_programs(sched)

    iota = np.broadcast_to(np.arange(128, dtype=np.float32), (128, 128)).astype(ml_dtypes.bfloat16)
    ident = np.eye(128, dtype=np.float32)
    iotaG = np.broadcast_to(np.arange(128, dtype=np.float32), (128, 128)).copy()
    xT = np.zeros((F, RPAD), dtype=ml_dtypes.bfloat16)
    xT[:, :N] = x.T.astype(ml_dtypes.bfloat16)
    W1 = np.asarray(W1, np.float32).astype(ml_dtypes.bfloat16)
    W2 = np.asarray(W2, np.float32).astype(ml_dtypes.bfloat16)
    Wmu = np.asarray(Wmu, np.float32); Wls = np.asarray(Wls, np.float32)
    B1 = np.broadcast_to(np.asarray(b1, np.float32), (128, H)).copy()
    B2 = np.broadcast_to(np.asarray(b2, np.float32), (128, H)).copy()
    Bmu = np.broadcast_to(np.asarray(bmu, np.float32), (128, Z)).copy()
    Bls = np.broadcast_to(np.asarray(bls, np.float32), (128, Z)).copy()

    ins1 = [dict(xT=xT, W1=W1, B1=B1, iota=iota, idx=cd["idx_w"],
                 dloc=cd["dloc_t"], cf=cd["cf_t"]) for cd in cores]
    kw = dict(trace=True) if _trace else {}
    res1 = run_bass_kernel_spmd(nc1, ins1, core_ids=list(range(NC)), **kw)

    h1 = np.zeros((N, H), dtype=np.float32)
    for c, cd in enumerate(cores):
        out = res1.results[c]["h1o"]
        for j in range(cd["nblk"]):
            b = cd["perm"][j]
            lo = cd["s"] + b * 128
            hi = min(lo + 128, cd["e"])
            h1[lo:hi] = out[j * 128 : j * 128 + (hi - lo)]

    h1T = np.zeros((H, RPAD), dtype=ml_dtypes.bfloat16)
    h1T[:, :N] = h1.T.astype(ml_dtypes.bfloat16)
    ins2 = [dict(h1T=h1T, W2=W2, B2=B2, iota=iota, idx=cd["idx_w"],
                 dloc=cd["dloc_t"], cf=cd["cf_t"], bloc=cd["bloc"],
                 iotaG=iotaG, invc=cd["invcnt"], ident=ident,
                 Wmu=Wmu, Wls=Wls, Bmu=Bmu, Bls=Bls) for cd in cores]
    res2 = run_bass_kernel_spmd(nc2, ins2, core_ids=list(range(NC)), **kw)

    mu = np.zeros((G, Z), dtype=np.float32)
    ls = np.zeros((G, Z), dtype=np.float32)
    for c, cd in enumerate(cores):
        ngr = cd["ge"] - cd["gs"]
        mu[cd["gs"] : cd["ge"]] = res2.results[c]["muo"][:ngr]
        ls[cd["gs"] : cd["ge"]] = res2.results[c]["lso"][:ngr]

    if _trace:
        kernel.last_exec_ns = (res1.exec_time_ns or 0) + (res2.exec_time_ns or 0)
        kernel.last_parts = (res1.exec_time_ns, res2.exec_time_ns)
    return mu, ls
